# revision 1
# baseline (speedup 1.0000x reference)
"""Trainium2 Bass kernel for Transformer-XL style relative-position attention.

Reference computation (B=2, Tq=1024, Tkv=2048, D=1024, H=16, Dv=64):
    q/k/v/r projections, ac = (q+cb)@k^T, bd = rel_shift((q+pb)@r^T),
    softmax((ac+bd)/8) with causal-with-memory mask, ctx = attn@v,
    out = LN(ctx@Wo + query).

Sharding (Megatron-style tensor parallel over heads, 8 cores):
  - each core owns 2 heads: Wq/Wk/Wv/Wr column shards [1024,128], biases
    shard with heads.
  - activations (transposed on host to feature-major) are broadcast.
  - after per-head attention, ctx^T shards are exchanged with a single
    AllToAll so that each core ends up with the full ctx^T for 1/8 of the
    token rows; each core then does that row-slice of ctx@Wo + residual +
    LayerNorm with the full Wo.

Key device-side tricks:
  - rel_shift is per-row: shifted[i, j] = raw[i, j + (Tq-1-i)].  We write
    raw bd tiles to a flat DRAM scratch and read them back through a
    strided AP (offset Tq-1, row stride Tkv-1), which lands the shifted
    rows back in SBUF with contiguous inner dim.
  - causal mask applied with gpsimd.affine_select (no mask tensor).
  - exp on ScalarE with accum_out gives softmax denominators for free.
  - attn is cast to bf16 and round-tripped through DRAM with a hardware
    DMA transpose so the attn@v contraction has TKV on partitions.
"""

import numpy as np

# problem shapes (hardcoded per contract)
B, TQ, TKV, D, NH, DV = 2, 1024, 2048, 1024, 16, 64
N_CORES = 8
HPC = NH // N_CORES          # heads per core = 2
FPC = HPC * DV               # head-feature columns per core = 128
RPC = (B * TQ) // N_CORES    # output token rows per core = 256
R_OFF = TKV - TQ             # causal memory offset = 1024
LN_EPS = 1e-5
NT = TQ // 128               # query row chunks = 8
NK = TKV // 512              # key col chunks of 512 = 4

_CACHE = {}


def _patched_tc_class():
    """TileContext whose kernel-tail drain splits sem waits one per drain.

    The walrus build in this container rejects CTRL-type instructions
    (InstDrain) carrying more than one sync-wait command.
    """
    import concourse.mybir as mybir
    import concourse.tile as tile
    from concourse.vector_clock import ScopedClock

    class TC(tile.TileContext):
        def _commit_instruction(self, inst, lazy_reg_writes=True):
            # This walrus build rejects instructions carrying more than one
            # sync-wait command; hoist extras onto preceding NoOp carriers.
            si = getattr(inst, "sync_info", None)
            if (
                si is not None
                and si.on_wait
                and len(si.on_wait) > 1
                and inst.engine != mybir.EngineType.Unassigned
            ):
                waits = list(si.on_wait)
                inst.sync_info = mybir.SyncInfo(
                    on_wait=[waits[-1]], on_update=list(si.on_update or [])
                )
                for w in waits[:-1]:
                    ev = mybir.InstNoOp(
                        name=f"I-wsplit-{self.nc.next_id()}", ins=[], outs=[]
                    )
                    ev.engine = inst.engine
                    ev.sync_info = mybir.SyncInfo(on_wait=[w], on_update=[])
                    self._add_instruction(ev)
            return super()._commit_instruction(inst, lazy_reg_writes)

        def _drain_and_barrier(self, tick_clock, wait_clock):
            nc = self.nc
            drain_inst = nc.sync.drain()
            wait_clock.add_sem_waits(
                drain_inst.ins, ScopedClock({None: tick_clock.global_clock})
            )
            inner = drain_inst.ins
            si = inner.sync_info
            waits = list(si.on_wait) if si and si.on_wait else []
            if len(waits) > 1:
                inner.sync_info = mybir.SyncInfo(
                    on_wait=waits[:1], on_update=list(si.on_update or [])
                )
                for w in waits[1:]:
                    d2 = nc.sync.drain()
                    d2.ins.sync_info = mybir.SyncInfo(on_wait=[w], on_update=[])
            nc.all_engine_barrier()
            assert self.sems is not None
            popped = nc._tile_sem_poison_stack.pop()
            assert popped is self._sem_poison
            nc.clear_and_free_semaphores(list(self.sems.allocated().values()))
            nc.all_engine_barrier()

    return TC


def build_program(score_dtype="bfloat16", proj_dtype="bfloat16", n_cores=N_CORES):
    """Build the SPMD Bass program (identical on all 8 cores).

    score_dtype / proj_dtype: "float32" (exact) or "float32r" (fast PE mode)
    for the score and projection matmuls respectively.
    n_cores=1 builds a single-core variant (collective over group [[0]])
    for profiling; its output is only valid for core 0's feature shard.
    """
    import concourse.bass as bass
    import concourse.mybir as mybir
    from concourse.bass import AP

    f32 = mybir.dt.float32
    bf16 = mybir.dt.bfloat16
    f32r = mybir.dt.float32r
    # matmul operand dtype is threaded through tensor dtypes: bfloat16 is the
    # fast PE path (1 cyc/row + fast weight load); float32r needs 2 serial
    # passes; float32 needs 4 cycles/row.
    DTM = {"float32": f32, "float32r": f32r, "bfloat16": bf16}
    pdt = DTM[proj_dtype]
    sdt = DTM[score_dtype]
    TC = _patched_tc_class()

    def mmcast(ap, dt_name):
        return ap

    nc = bass.Bass()

    # ---- I/O ----
    xqT = nc.dram_tensor("xqT", [D, B * TQ], pdt, kind="ExternalInput")
    xkvT = nc.dram_tensor("xkvT", [D, B * TKV], pdt, kind="ExternalInput")
    xrT = nc.dram_tensor("xrT", [D, B * TKV], pdt, kind="ExternalInput")
    wq = nc.dram_tensor("wq", [D, FPC], pdt, kind="ExternalInput")
    wk = nc.dram_tensor("wk", [D, FPC], pdt, kind="ExternalInput")
    wv = nc.dram_tensor("wv", [D, FPC], pdt, kind="ExternalInput")
    wr = nc.dram_tensor("wr", [D, FPC], pdt, kind="ExternalInput")
    wo = nc.dram_tensor("wo", [D, D], pdt, kind="ExternalInput")
    cbv = nc.dram_tensor("cbv", [FPC, 1], f32, kind="ExternalInput")
    pbv = nc.dram_tensor("pbv", [FPC, 1], f32, kind="ExternalInput")
    qres = nc.dram_tensor("qres", [RPC, D], f32, kind="ExternalInput")
    gamma = nc.dram_tensor("gamma", [D], f32, kind="ExternalInput")
    beta = nc.dram_tensor("beta", [D], f32, kind="ExternalInput")
    out = nc.dram_tensor("out", [RPC, D], f32, kind="ExternalOutput")

    # ---- internal DRAM scratch ----
    bd_dram = [nc.dram_tensor(f"bd_dram{p}", [TQ * TKV], bf16) for p in range(4)]
    attn_dram = [nc.dram_tensor(f"attn_dram{p}", [TQ, TKV], bf16) for p in range(4)]
    a2a_in = nc.dram_tensor("a2a_in", [N_CORES * FPC, RPC], pdt)
    a2a_out = nc.dram_tensor("a2a_out", [N_CORES * FPC, RPC], pdt)

    Exp = mybir.ActivationFunctionType.Exp
    Identity = mybir.ActivationFunctionType.Identity
    Sqrt = mybir.ActivationFunctionType.Sqrt
    ALU = mybir.AluOpType

    with TC(nc) as tc:
        import contextlib

        with contextlib.ExitStack() as ctx:
            singles = ctx.enter_context(tc.tile_pool(name="singles", bufs=1))

            # ---- static SBUF tensors ----
            wq_sb = singles.tile([128, D // 128, FPC], pdt, tag="wq_sb")
            wk_sb = singles.tile([128, D // 128, FPC], pdt, tag="wk_sb")
            wv_sb = singles.tile([128, D // 128, FPC], pdt, tag="wv_sb")
            wr_sb = singles.tile([128, D // 128, FPC], pdt, tag="wr_sb")
            for w_sb, w_dr in ((wq_sb, wq), (wk_sb, wk), (wv_sb, wv), (wr_sb, wr)):
                nc.sync.dma_start(
                    out=w_sb, in_=w_dr[:].rearrange("(kc p) f -> p kc f", p=128)
                )
            wo_sb = singles.tile([128, D // 128, D], pdt, tag="wo_sb")
            nc.gpsimd.dma_start(
                out=wo_sb, in_=wo[:].rearrange("(kc p) d -> p kc d", p=128)
            )
            cb_sb = singles.tile([FPC, 1], f32, tag="cb_sb")
            pb_sb = singles.tile([FPC, 1], f32, tag="pb_sb")
            nc.sync.dma_start(out=cb_sb, in_=cbv[:])
            nc.sync.dma_start(out=pb_sb, in_=pbv[:])
            eps_sb = singles.tile([128, 1], f32, tag="eps_sb")
            nc.vector.memset(eps_sb, LN_EPS)
            gamma_sb = singles.tile([128, D], f32, tag="gamma_sb")
            beta_sb = singles.tile([128, D], f32, tag="beta_sb")
            nc.gpsimd.dma_start(
                out=gamma_sb,
                in_=AP(tensor=gamma[:].tensor, offset=0, ap=[[0, 128], [1, D]]),
            )
            nc.gpsimd.dma_start(
                out=beta_sb,
                in_=AP(tensor=beta[:].tensor, offset=0, ap=[[0, 128], [1, D]]),
            )
            qres_sb = singles.tile([128, RPC // 128, D], f32, tag="qres_sb")
            nc.gpsimd.dma_start(
                out=qres_sb, in_=qres[:].rearrange("(mc p) d -> p mc d", p=128)
            )

            # projection outputs (feature-major, both heads stacked on partitions)
            qcb_sb = singles.tile([FPC, B * TQ], sdt, tag="qcb_sb")
            qpb_sb = singles.tile([FPC, B * TQ], sdt, tag="qpb_sb")
            kT_sb = singles.tile([FPC, B * TKV], sdt, tag="kT_sb")
            rT_sb = singles.tile([FPC, B * TKV], sdt, tag="rT_sb")
            v_sb = singles.tile([128, (B * TKV) // 128, FPC], bf16, tag="v_sb")
            ctx_sb = singles.tile([FPC, B * TQ], pdt, tag="ctx_sb")

            # identity (bf16) for PE-transposes and PSUM-accumulate adds
            ident_bf = singles.tile([128, 128], bf16, tag="ident_bf")
            ident_f32 = singles.tile([128, 128], f32, tag="ident_f32")
            from concourse.masks import make_identity

            make_identity(nc, ident_bf)
            make_identity(nc, ident_f32)

            # ========== Phases A+B interleaved: projections + attention ==========
            # batch-0 projections, then batch-0 pairs (their PE work fills the
            # DMA-bound projection gaps of batch 1), then batch-1 pairs.
            CH = 512  # token columns per projection step
            with tc.tile_pool(name="pb_rows", bufs=2) as pb_rows, tc.tile_pool(
                name="pb_t", bufs=3
            ) as pb_t, tc.tile_pool(name="pb_small", bufs=6) as pb_small:
                pa_in = tc.alloc_tile_pool(name="pa_in", bufs=2)
                pa_ps = tc.alloc_tile_pool(name="pa_ps", bufs=4, space="PSUM")
                pa_psv = tc.alloc_tile_pool(name="pa_psv", bufs=4, space="PSUM")

                def emit_q_chunk(j):
                    q_in = pa_in.tile(
                        [128, D // 128, CH], pdt, tag="xin", name=f"q_in{j}"
                    )
                    nc.sync.dma_start(
                        out=q_in,
                        in_=xqT[:].rearrange("(kc p) t -> p kc t", p=128)[
                            :, :, j * CH : (j + 1) * CH
                        ],
                    )
                    ps = pa_ps.tile([FPC, CH], f32, tag="ps", name=f"ps_q{j}")
                    for kc in range(D // 128):
                        nc.tensor.matmul(
                            ps,
                            wq_sb[:, kc, :],
                            q_in[:, kc, :],
                            start=(kc == 0),
                            stop=(kc == D // 128 - 1),
                        )
                    sl = slice(j * CH, (j + 1) * CH)
                    nc.scalar.activation(
                        out=qcb_sb[:, sl], in_=ps, func=Identity, bias=cb_sb
                    )
                    nc.scalar.activation(
                        out=qpb_sb[:, sl], in_=ps, func=Identity, bias=pb_sb
                    )

                def emit_kvr_chunk(j):
                    kv_in = pa_in.tile(
                        [128, D // 128, CH], pdt, tag="xin", name=f"kv_in{j}"
                    )
                    nc.sync.dma_start(
                        out=kv_in,
                        in_=xkvT[:].rearrange("(kc p) t -> p kc t", p=128)[
                            :, :, j * CH : (j + 1) * CH
                        ],
                    )
                    ps = pa_ps.tile([FPC, CH], f32, tag="ps", name=f"ps_k{j}")
                    for kc in range(D // 128):
                        nc.tensor.matmul(
                            ps,
                            wk_sb[:, kc, :],
                            kv_in[:, kc, :],
                            start=(kc == 0),
                            stop=(kc == D // 128 - 1),
                        )
                    sl = slice(j * CH, (j + 1) * CH)
                    nc.scalar.copy(out=kT_sb[:, sl], in_=ps)
                    # v: compute vT (feature-major, fast N) then PE-transpose
                    # into natural [tokens, feats] bf16 tiles
                    psvt = pa_ps.tile([FPC, CH], f32, tag="ps", name=f"psvt{j}")
                    for kc in range(D // 128):
                        nc.tensor.matmul(
                            psvt,
                            wv_sb[:, kc, :],
                            kv_in[:, kc, :],
                            start=(kc == 0),
                            stop=(kc == D // 128 - 1),
                        )
                    vt_t = pa_in.tile([FPC, CH], pdt, tag="vt_t", name=f"vt_t{j}")
                    nc.scalar.copy(out=vt_t, in_=psvt)
                    for s in range(CH // 128):
                        psv = pa_psv.tile([128, FPC], pdt, tag="psv", name=f"psv{j}_{s}")
                        nc.tensor.transpose(
                            psv,
                            vt_t[:, s * 128 : (s + 1) * 128],
                            ident_bf if pdt == bf16 else ident_f32,
                        )
                        nc.scalar.copy(out=v_sb[:, j * (CH // 128) + s, :], in_=psv)
                    r_in = pa_in.tile(
                        [128, D // 128, CH], pdt, tag="xin2", name=f"r_in{j}"
                    )
                    nc.scalar.dma_start(
                        out=r_in,
                        in_=xrT[:].rearrange("(kc p) t -> p kc t", p=128)[
                            :, :, j * CH : (j + 1) * CH
                        ],
                    )
                    ps2 = pa_ps.tile([FPC, CH], f32, tag="ps", name=f"ps_r{j}")
                    for kc in range(D // 128):
                        nc.tensor.matmul(
                            ps2,
                            wr_sb[:, kc, :],
                            r_in[:, kc, :],
                            start=(kc == 0),
                            stop=(kc == D // 128 - 1),
                        )
                    nc.scalar.copy(out=rT_sb[:, sl], in_=ps2)

                def emit_ctx(pi, b, qf):
                    ps_cx = [
                        pb_ctx.tile([64, 512], f32, tag="ps_cx", name=f"ps_cx{pi}_{n_}")
                        for n_ in range(2)
                    ]
                    for kc in range(TKV // 128):
                        attnT = pb_t.tile([128, TQ], bf16, tag="attnT")
                        nc.sync.dma_start(
                            out=attnT,
                            in_=attn_dram[pi][:, kc * 128 : (kc + 1) * 128],
                            transpose=True,
                        )
                        for nn in range(2):
                            if nn == 0 and kc >= 12:
                                continue
                            nc.tensor.matmul(
                                ps_cx[nn],
                                v_sb[:, b * (TKV // 128) + kc, qf],
                                attnT[:, nn * 512 : (nn + 1) * 512],
                                start=(kc == 0),
                                stop=(kc == (11 if nn == 0 else TKV // 128 - 1)),
                            )
                    for nn in range(2):
                        nc.vector.tensor_copy(
                            out=ctx_sb[qf, b * TQ + nn * 512 : b * TQ + (nn + 1) * 512],
                            in_=ps_cx[nn],
                        )

                def emit_pair(pi, b, hh):
                    qf = slice(64 * hh, 64 * hh + 64)
                    # pass 1: raw bd rows (dense PE work) -> DRAM scratch
                    for t in range(NT):
                        n0 = 1 if t < 4 else 0
                        bd_row = pb_rows.tile([128, TKV], bf16, tag="bd_row")
                        for n in range(n0, NK):
                            ps_bd = pb_ps.tile([128, 512], f32, tag="ps_sc")
                            nc.tensor.matmul(
                                ps_bd,
                                qpb_sb[qf, t * 128 : (t + 1) * 128],
                                rT_sb[qf, b * TKV + 512 * n : b * TKV + 512 * (n + 1)],
                                start=True,
                                stop=True,
                            )
                            if n % 2 == 0:
                                nc.scalar.copy(
                                    out=bd_row[:, 512 * n : 512 * (n + 1)], in_=ps_bd
                                )
                            else:
                                nc.vector.tensor_copy(
                                    out=bd_row[:, 512 * n : 512 * (n + 1)], in_=ps_bd
                                )
                        nc.scalar.dma_start(
                            out=AP(
                                tensor=bd_dram[pi][:].tensor,
                                offset=t * 128 * TKV + 512 * n0,
                                ap=[[TKV, 128], [1, TKV - 512 * n0]],
                            ),
                            in_=bd_row[:, 512 * n0 : TKV],
                        )
                    # pass 2: shifted reads + ac + softmax; ctx halves start
                    # as soon as their query rows are in DRAM
                    for t in range(NT):
                        n_last = 2 + t // 4
                        ncols = 512 * (n_last + 1)
                        bd_shift = pb_rows.tile([128, TKV], bf16, tag="bd_shift")
                        nc.scalar.dma_start(
                            out=bd_shift[:, :ncols],
                            in_=AP(
                                tensor=bd_dram[pi][:].tensor,
                                offset=t * 128 * (TKV - 1) + (TQ - 1),
                                ap=[[TKV - 1, 128], [1, ncols]],
                            ),
                        )
                        nc.gpsimd.affine_select(
                            out=bd_shift[:, 512 * n_last : 512 * (n_last + 1)],
                            in_=bd_shift[:, 512 * n_last : 512 * (n_last + 1)],
                            pattern=[[-1, 512]],
                            compare_op=ALU.is_ge,
                            fill=-1e30,
                            base=R_OFF + 128 * t - 512 * n_last,
                            channel_multiplier=1,
                        )
                        attn_row = pb_t.tile([128, TKV], bf16, tag="attn_row")
                        rowsum = pb_small.tile([128, 2, 1], f32, tag="rowsum")
                        for h in range(2):
                            hw = min(ncols - 1024 * h, 1024)
                            ps_ac = pb_ps2.tile([128, 1024], f32, tag="ps_ac")
                            for nn2 in range(hw // 512):
                                n = 2 * h + nn2
                                psl = slice(512 * nn2, 512 * (nn2 + 1))
                                nc.tensor.matmul(
                                    ps_ac[:, psl],
                                    qcb_sb[qf, t * 128 : (t + 1) * 128],
                                    kT_sb[qf, b * TKV + 512 * n : b * TKV + 512 * (n + 1)],
                                    start=True,
                                    stop=False,
                                )
                                nc.tensor.matmul(
                                    ps_ac[:, psl],
                                    ident_bf,
                                    bd_shift[:, 512 * n : 512 * (n + 1)],
                                    start=False,
                                    stop=True,
                                )
                            nc.scalar.activation(
                                out=attn_row[:, 1024 * h : 1024 * h + hw],
                                in_=ps_ac[:, :hw],
                                func=Exp,
                                scale=0.125,
                                accum_out=rowsum[:, h, :],
                            )
                        nc.vector.tensor_add(
                            out=rowsum[:, 0, :],
                            in0=rowsum[:, 0, :],
                            in1=rowsum[:, 1, :],
                        )
                        recip = pb_small.tile([128, 1], f32, tag="recip")
                        nc.vector.reciprocal(recip, rowsum[:, 0, :])
                        nc.vector.tensor_scalar_mul(
                            out=attn_row[:, :ncols],
                            in0=attn_row[:, :ncols],
                            scalar1=recip,
                        )
                        if ncols < TKV:
                            nc.vector.memset(attn_row[:, ncols:], 0.0)
                        nc.gpsimd.dma_start(
                            out=attn_dram[pi][t * 128 : (t + 1) * 128, :],
                            in_=attn_row,
                        )
                    emit_ctx(pi, b, qf)

                for j in range(2):
                    emit_q_chunk(j)
                for j in range(4):
                    emit_kvr_chunk(j)
                for j in range(2, 4):
                    emit_q_chunk(j)
                for j in range(4, 8):
                    emit_kvr_chunk(j)
                pa_psv.release()
                pa_ps.release()
                pa_in.release()
                pb_ps = tc.alloc_tile_pool(name="pb_ps", bufs=2, space="PSUM")
                pb_ps2 = tc.alloc_tile_pool(name="pb_ps2", bufs=2, space="PSUM")
                pb_ctx = tc.alloc_tile_pool(name="pb_ctx", bufs=2, space="PSUM")
                # zero-init bd scratch regions the shifted reads can touch but
                # raw writes never cover (masked spill into next row's low
                # columns): rows 0..511 cols 0..511 + first row of chunks 5..7
                zeros_bf = pb_t.tile([128, 512], bf16, tag="zeros_bf")
                nc.vector.memset(zeros_bf, 0.0)
                for pi in range(4):
                    for rc in range(4):
                        nc.gpsimd.dma_start(
                            out=AP(
                                tensor=bd_dram[pi][:].tensor,
                                offset=rc * 128 * TKV,
                                ap=[[TKV, 128], [1, 512]],
                            ),
                            in_=zeros_bf,
                        )
                    nc.gpsimd.dma_start(
                        out=AP(
                            tensor=bd_dram[pi][:].tensor,
                            offset=5 * 128 * TKV,
                            ap=[[128 * TKV, 3], [1, 512]],
                        ),
                        in_=zeros_bf[0:3, :],
                    )

                emit_pair(0, 0, 0)
                emit_pair(1, 0, 1)
                emit_pair(2, 1, 0)
                emit_pair(3, 1, 1)
                pb_ctx.release()
                pb_ps2.release()
                pb_ps.release()

            # ================= Phase C: exchange + output projection + LN =========
            nc.sync.dma_start(
                out=a2a_in[:].rearrange("(j p) t -> p j t", p=FPC),
                in_=ctx_sb[:].rearrange("p (j t) -> p j t", t=RPC),
            )
            if n_cores > 1:
                nc.gpsimd.collective_compute(
                    "AllToAll",
                    ALU.bypass,
                    replica_groups=[list(range(n_cores))],
                    ins=[a2a_in[:]],
                    outs=[a2a_out[:]],
                )
            else:
                # single-core profiling variant: plain copy instead
                nc.sync.dma_start(out=a2a_out[:], in_=a2a_in[:])
            with tc.tile_pool(name="pc", bufs=3) as pc, tc.tile_pool(
                name="pc_ps", bufs=4, space="PSUM"
            ) as pc_ps, tc.tile_pool(name="pc_small", bufs=8) as pc_small:
                for mc in range(RPC // 128):
                    ps_o = [pc_ps.tile([128, 512], f32, tag="ps_o", name=f"ps_o{nn_}") for nn_ in range(2)]
                    for kc in range(D // 128):
                        lhs = pc.tile([128, 128], pdt, tag="octx")
                        nc.sync.dma_start(
                            out=lhs,
                            in_=a2a_out[
                                kc * 128 : (kc + 1) * 128, mc * 128 : (mc + 1) * 128
                            ],
                        )
                        for nn in range(2):
                            nc.tensor.matmul(
                                ps_o[nn],
                                lhs,
                                wo_sb[:, kc, nn * 512 : (nn + 1) * 512],
                                start=(kc == 0),
                                stop=(kc == D // 128 - 1),
                            )
                    o_sb = pc.tile([128, D], f32, tag="o_sb")
                    for nn in range(2):
                        nc.vector.tensor_add(
                            out=o_sb[:, nn * 512 : (nn + 1) * 512],
                            in0=ps_o[nn],
                            in1=qres_sb[:, mc, nn * 512 : (nn + 1) * 512],
                        )
                    # LayerNorm over the free (feature) dim
                    stats = pc_small.tile([128, 2, 6], f32, tag="stats")
                    for sg in range(2):
                        nc.vector.bn_stats(
                            out=stats[:, sg, :], in_=o_sb[:, sg * 512 : (sg + 1) * 512]
                        )
                    mv = pc_small.tile([128, 2], f32, tag="mv")
                    nc.vector.bn_aggr(out=mv, in_=stats)
                    mean, var = mv[:, 0:1], mv[:, 1:2]
                    xve = pc_small.tile([128, 1], f32, tag="xve")
                    nc.vector.tensor_scalar_add(out=xve, in0=var, scalar1=eps_sb)
                    std = pc_small.tile([128, 1], f32, tag="std")
                    nc.scalar.activation(out=std, in_=var, func=Sqrt, bias=eps_sb)
                    rstd = pc_small.tile([128, 1], f32, tag="rstd")
                    nc.vector.reciprocal(rstd, std)
                    # one Newton step for rsqrt accuracy:
                    # r <- r * (1.5 - 0.5 * x * r^2)
                    tnw = pc_small.tile([128, 1], f32, tag="tnw")
                    nc.vector.tensor_mul(out=tnw, in0=rstd, in1=rstd)
                    nc.vector.tensor_mul(out=tnw, in0=tnw, in1=xve)
                    nc.vector.tensor_scalar(
                        out=tnw, in0=tnw, scalar1=-0.5, scalar2=1.5,
                        op0=ALU.mult, op1=ALU.add,
                    )
                    nc.vector.tensor_scalar_mul(out=rstd, in0=rstd, scalar1=tnw)
                    nc.vector.tensor_scalar(
                        out=o_sb, in0=o_sb, scalar1=mean, scalar2=rstd,
                        op0=ALU.subtract, op1=ALU.mult,
                    )
                    nc.vector.tensor_mul(out=o_sb, in0=o_sb, in1=gamma_sb)
                    nc.vector.tensor_add(out=o_sb, in0=o_sb, in1=beta_sb)
                    nc.sync.dma_start(
                        out=out[mc * 128 : (mc + 1) * 128, :], in_=o_sb
                    )
    return nc


def _make_in_maps(inputs, mm_dtype="bfloat16"):
    query = np.asarray(inputs["query"], np.float32)
    key_value = np.asarray(inputs["key_value"], np.float32)
    relative = np.asarray(inputs["relative"], np.float32)
    content_bias = np.asarray(inputs["content_bias"], np.float32)
    position_bias = np.asarray(inputs["position_bias"], np.float32)
    Wq, Wk = np.asarray(inputs["Wq"], np.float32), np.asarray(inputs["Wk"], np.float32)
    Wv, Wr = np.asarray(inputs["Wv"], np.float32), np.asarray(inputs["Wr"], np.float32)
    Wo = np.ascontiguousarray(np.asarray(inputs["Wo"], np.float32))
    ln_gamma = np.asarray(inputs["ln_gamma"], np.float32)
    ln_beta = np.asarray(inputs["ln_beta"], np.float32)

    qflat = query.reshape(B * TQ, D)
    if mm_dtype == "bfloat16":
        import ml_dtypes

        mdt = ml_dtypes.bfloat16
    else:
        mdt = np.float32
    xqT = np.ascontiguousarray(qflat.T).astype(mdt)
    xkvT = np.ascontiguousarray(key_value.reshape(B * TKV, D).T).astype(mdt)
    xrT = np.ascontiguousarray(relative.reshape(B * TKV, D).T).astype(mdt)
    Wq, Wk = Wq.astype(mdt), Wk.astype(mdt)
    Wv, Wr = Wv.astype(mdt), Wr.astype(mdt)
    Wo = Wo.astype(mdt)
    cb = content_bias.reshape(NH, DV)
    pb = position_bias.reshape(NH, DV)

    in_maps = []
    for c in range(N_CORES):
        fs = slice(FPC * c, FPC * (c + 1))
        in_maps.append(
            {
                "xqT": xqT,
                "xkvT": xkvT,
                "xrT": xrT,
                "wq": np.ascontiguousarray(Wq[:, fs]),
                "wk": np.ascontiguousarray(Wk[:, fs]),
                "wv": np.ascontiguousarray(Wv[:, fs]),
                "wr": np.ascontiguousarray(Wr[:, fs]),
                "wo": Wo,
                "cbv": np.ascontiguousarray(
                    cb[HPC * c : HPC * (c + 1)].reshape(FPC, 1)
                ),
                "pbv": np.ascontiguousarray(
                    pb[HPC * c : HPC * (c + 1)].reshape(FPC, 1)
                ),
                "qres": np.ascontiguousarray(qflat[RPC * c : RPC * (c + 1)]),
                "gamma": ln_gamma,
                "beta": ln_beta,
            }
        )
    return in_maps


def run_on_hw(inputs, trace=False, score_dtype="bfloat16", proj_dtype="bfloat16"):
    from concourse.bass_utils import run_bass_kernel_spmd

    key = (score_dtype, proj_dtype)
    nc = _CACHE.get(key)
    if nc is None:
        nc = build_program(score_dtype=score_dtype, proj_dtype=proj_dtype)
        _CACHE[key] = nc
    in_maps = _make_in_maps(inputs, mm_dtype=proj_dtype)
    res = run_bass_kernel_spmd(nc, in_maps, list(range(N_CORES)), trace=trace)
    outs = np.concatenate(
        [np.asarray(res.results[c]["out"]) for c in range(N_CORES)], axis=0
    )
    return outs.reshape(B, TQ, D), res


def kernel(**inputs) -> np.ndarray:
    out, _ = run_on_hw(inputs)
    return out



# revision 4
# speedup vs baseline: 1.1802x; 1.1802x over previous
"""Trainium2 Bass kernel for Transformer-XL style relative-position attention.

Reference computation (B=2, Tq=1024, Tkv=2048, D=1024, H=16, Dv=64):
    q/k/v/r projections, ac = (q+cb)@k^T, bd = rel_shift((q+pb)@r^T),
    softmax((ac+bd)/8) with causal-with-memory mask, ctx = attn@v,
    out = LN(ctx@Wo + query).

Sharding (Megatron-style tensor parallel over heads, 8 cores):
  - each core owns 2 heads: Wq/Wk/Wv/Wr column shards [1024,128], biases
    shard with heads.
  - activations (transposed on host to feature-major) are broadcast.
  - after per-head attention, ctx^T shards are exchanged with a single
    AllToAll so that each core ends up with the full ctx^T for 1/8 of the
    token rows; each core then does that row-slice of ctx@Wo + residual +
    LayerNorm with the full Wo.

Device-side structure (transposed-scores design):
  - scores are computed TRANSPOSED (kv on partitions, q on the free dim):
    acT tiles come straight from a matmul with kT as the stationary
    operand; this makes the softmax output directly consumable by the
    attn@v contraction with NO attention-matrix transpose or DRAM
    round-trip.
  - rel_shift: raw bd is computed q-major (dense PE work), written to a
    flat DRAM scratch, and read back through a strided AP with row
    stride Tkv-1 PLUS transpose=True (hardware XBAR transpose), which
    lands the *shifted, transposed* bd tiles in SBUF in one step.
  - causal mask applied to the bd tiles with gpsimd.affine_select.
  - softmax denominators come for free from a ones-column appended to v:
    the attn@v matmul accumulates sum(exp) in psum row 64.
  - 1/denominator (per q) is broadcast across the 64 feature partitions
    with a rank-1 matmul (ones ⊗ recip) and applied to the small ctx^T
    tile instead of the big attention matrix.
"""

import numpy as np

# problem shapes (hardcoded per contract)
B, TQ, TKV, D, NH, DV = 2, 1024, 2048, 1024, 16, 64
N_CORES = 8
HPC = NH // N_CORES          # heads per core = 2
FPC = HPC * DV               # head-feature columns per core = 128
RPC = (B * TQ) // N_CORES    # output token rows per core = 256
R_OFF = TKV - TQ             # causal memory offset = 1024
LN_EPS = 1e-5
NT = TQ // 128               # query row chunks = 8
NK = TKV // 512              # key col chunks of 512 = 4

_CACHE = {}


def _patched_tc_class():
    """TileContext whose kernel-tail drain splits sem waits one per drain.

    The walrus build in this container rejects CTRL-type instructions
    (InstDrain) carrying more than one sync-wait command.
    """
    import concourse.mybir as mybir
    import concourse.tile as tile
    from concourse.vector_clock import ScopedClock

    class TC(tile.TileContext):
        def _commit_instruction(self, inst, lazy_reg_writes=True):
            # This walrus build rejects instructions carrying more than one
            # sync-wait command; hoist extras onto preceding NoOp carriers.
            si = getattr(inst, "sync_info", None)
            if (
                si is not None
                and si.on_wait
                and len(si.on_wait) > 1
                and inst.engine != mybir.EngineType.Unassigned
            ):
                waits = list(si.on_wait)
                inst.sync_info = mybir.SyncInfo(
                    on_wait=[waits[-1]], on_update=list(si.on_update or [])
                )
                for w in waits[:-1]:
                    ev = mybir.InstNoOp(
                        name=f"I-wsplit-{self.nc.next_id()}", ins=[], outs=[]
                    )
                    ev.engine = inst.engine
                    ev.sync_info = mybir.SyncInfo(on_wait=[w], on_update=[])
                    self._add_instruction(ev)
            return super()._commit_instruction(inst, lazy_reg_writes)

        def _drain_and_barrier(self, tick_clock, wait_clock):
            nc = self.nc
            drain_inst = nc.sync.drain()
            wait_clock.add_sem_waits(
                drain_inst.ins, ScopedClock({None: tick_clock.global_clock})
            )
            inner = drain_inst.ins
            si = inner.sync_info
            waits = list(si.on_wait) if si and si.on_wait else []
            if len(waits) > 1:
                inner.sync_info = mybir.SyncInfo(
                    on_wait=waits[:1], on_update=list(si.on_update or [])
                )
                for w in waits[1:]:
                    d2 = nc.sync.drain()
                    d2.ins.sync_info = mybir.SyncInfo(on_wait=[w], on_update=[])
            nc.all_engine_barrier()
            assert self.sems is not None
            popped = nc._tile_sem_poison_stack.pop()
            assert popped is self._sem_poison
            nc.clear_and_free_semaphores(list(self.sems.allocated().values()))
            nc.all_engine_barrier()

    return TC


def build_program(score_dtype="bfloat16", proj_dtype="bfloat16", n_cores=N_CORES):
    """Build the SPMD Bass program (identical on all 8 cores).

    n_cores=1 builds a single-core variant (collective replaced by a
    self-copy) for profiling; its output is only valid for core 0's
    feature shard.
    """
    import concourse.bass as bass
    import concourse.mybir as mybir
    from concourse.bass import AP

    f32 = mybir.dt.float32
    bf16 = mybir.dt.bfloat16
    pdt = bf16
    sdt = bf16
    TC = _patched_tc_class()

    nc = bass.Bass()

    # ---- I/O ----
    xqT = nc.dram_tensor("xqT", [D, B * TQ], pdt, kind="ExternalInput")
    xkvT = nc.dram_tensor("xkvT", [D, B * TKV], pdt, kind="ExternalInput")
    xrT = nc.dram_tensor("xrT", [D, B * TKV], pdt, kind="ExternalInput")
    wq = nc.dram_tensor("wq", [D, FPC], pdt, kind="ExternalInput")
    wk = nc.dram_tensor("wk", [D, FPC], pdt, kind="ExternalInput")
    wv = nc.dram_tensor("wv", [D, FPC], pdt, kind="ExternalInput")
    wr = nc.dram_tensor("wr", [D, FPC], pdt, kind="ExternalInput")
    wo = nc.dram_tensor("wo", [D, D], pdt, kind="ExternalInput")
    cbv = nc.dram_tensor("cbv", [FPC, 1], f32, kind="ExternalInput")
    pbv = nc.dram_tensor("pbv", [FPC, 1], f32, kind="ExternalInput")
    qres = nc.dram_tensor("qres", [RPC, D], f32, kind="ExternalInput")
    gamma = nc.dram_tensor("gamma", [D], f32, kind="ExternalInput")
    beta = nc.dram_tensor("beta", [D], f32, kind="ExternalInput")
    out = nc.dram_tensor("out", [RPC, D], f32, kind="ExternalOutput")

    # ---- internal DRAM scratch ----
    # raw bd per (pair, q-half): flat [512 rows x TKV]; the shifted,
    # transposed read only ever depends on its own half's rows.
    bd_dram = [
        [nc.dram_tensor(f"bd_dram{p}_{h}", [512 * TKV], bf16) for h in range(2)]
        for p in range(4)
    ]
    a2a_in = nc.dram_tensor("a2a_in", [N_CORES * FPC, RPC], pdt)
    a2a_out = nc.dram_tensor("a2a_out", [N_CORES * FPC, RPC], pdt)

    Exp = mybir.ActivationFunctionType.Exp
    Identity = mybir.ActivationFunctionType.Identity
    Sqrt = mybir.ActivationFunctionType.Sqrt
    ALU = mybir.AluOpType

    with TC(nc) as tc:
        import contextlib

        with contextlib.ExitStack() as ctx:
            singles = ctx.enter_context(tc.tile_pool(name="singles", bufs=1))

            # ---- static SBUF tensors ----
            wq_sb = singles.tile([128, D // 128, FPC], pdt, tag="wq_sb")
            wk_sb = singles.tile([128, D // 128, FPC], pdt, tag="wk_sb")
            wv_sb = singles.tile([128, D // 128, FPC], pdt, tag="wv_sb")
            wr_sb = singles.tile([128, D // 128, FPC], pdt, tag="wr_sb")
            for w_sb, w_dr in ((wq_sb, wq), (wk_sb, wk), (wv_sb, wv), (wr_sb, wr)):
                nc.sync.dma_start(
                    out=w_sb, in_=w_dr[:].rearrange("(kc p) f -> p kc f", p=128)
                )
            wo_sb = singles.tile([128, D // 128, D], pdt, tag="wo_sb")
            nc.gpsimd.dma_start(
                out=wo_sb, in_=wo[:].rearrange("(kc p) d -> p kc d", p=128)
            )
            cb_sb = singles.tile([FPC, 1], f32, tag="cb_sb")
            pb_sb = singles.tile([FPC, 1], f32, tag="pb_sb")
            nc.sync.dma_start(out=cb_sb, in_=cbv[:])
            nc.sync.dma_start(out=pb_sb, in_=pbv[:])
            eps_sb = singles.tile([128, 1], f32, tag="eps_sb")
            nc.vector.memset(eps_sb, LN_EPS)
            gamma_sb = singles.tile([128, D], f32, tag="gamma_sb")
            beta_sb = singles.tile([128, D], f32, tag="beta_sb")
            nc.gpsimd.dma_start(
                out=gamma_sb,
                in_=AP(tensor=gamma[:].tensor, offset=0, ap=[[0, 128], [1, D]]),
            )
            nc.gpsimd.dma_start(
                out=beta_sb,
                in_=AP(tensor=beta[:].tensor, offset=0, ap=[[0, 128], [1, D]]),
            )
            qres_sb = singles.tile([128, RPC // 128, D], f32, tag="qres_sb")
            nc.gpsimd.dma_start(
                out=qres_sb, in_=qres[:].rearrange("(mc p) d -> p mc d", p=128)
            )

            # projection outputs (feature-major, both heads stacked on partitions)
            qcb_sb = singles.tile([FPC, B * TQ], sdt, tag="qcb_sb")
            qpb_sb = singles.tile([FPC, B * TQ], sdt, tag="qpb_sb")
            kT_sb = singles.tile([FPC, B * TKV], sdt, tag="kT_sb")
            rT_sb = singles.tile([FPC, B * TKV], sdt, tag="rT_sb")
            # v in natural layout [kv-token partitions, chunk, head, 64+ones]
            v_sb = singles.tile([128, (B * TKV) // 128, HPC, DV + 1], bf16, tag="v_sb")
            nc.vector.memset(v_sb[:, :, :, DV], 1.0)
            ctx_sb = singles.tile([FPC, B * TQ], pdt, tag="ctx_sb")
            ones_bf = singles.tile([1, DV], bf16, tag="ones_bf")
            nc.vector.memset(ones_bf, 1.0)

            # identity (bf16) for PE-transposes
            ident_bf = singles.tile([128, 128], bf16, tag="ident_bf")
            from concourse.masks import make_identity

            make_identity(nc, ident_bf)

            # ========== Phases A+B interleaved: projections + attention ==========
            CH = 512  # token columns per projection step
            with contextlib.ExitStack() as phase_ab:
                pa_in = tc.alloc_tile_pool(name="pa_in", bufs=2)
                pa_ps = tc.alloc_tile_pool(name="pa_ps", bufs=4, space="PSUM")
                pa_psv = tc.alloc_tile_pool(name="pa_psv", bufs=4, space="PSUM")

                def emit_q_chunk(j):
                    q_in = pa_in.tile(
                        [128, D // 128, CH], pdt, tag="xin", name=f"q_in{j}"
                    )
                    nc.sync.dma_start(
                        out=q_in,
                        in_=xqT[:].rearrange("(kc p) t -> p kc t", p=128)[
                            :, :, j * CH : (j + 1) * CH
                        ],
                    )
                    ps = pa_ps.tile([FPC, CH], f32, tag="ps", name=f"ps_q{j}")
                    for kc in range(D // 128):
                        nc.tensor.matmul(
                            ps,
                            wq_sb[:, kc, :],
                            q_in[:, kc, :],
                            start=(kc == 0),
                            stop=(kc == D // 128 - 1),
                        )
                    sl = slice(j * CH, (j + 1) * CH)
                    nc.scalar.activation(
                        out=qcb_sb[:, sl], in_=ps, func=Identity, bias=cb_sb
                    )
                    nc.scalar.activation(
                        out=qpb_sb[:, sl], in_=ps, func=Identity, bias=pb_sb
                    )

                def emit_kvr_chunk(j):
                    kv_in = pa_in.tile(
                        [128, D // 128, CH], pdt, tag="xin", name=f"kv_in{j}"
                    )
                    nc.sync.dma_start(
                        out=kv_in,
                        in_=xkvT[:].rearrange("(kc p) t -> p kc t", p=128)[
                            :, :, j * CH : (j + 1) * CH
                        ],
                    )
                    ps = pa_ps.tile([FPC, CH], f32, tag="ps", name=f"ps_k{j}")
                    for kc in range(D // 128):
                        nc.tensor.matmul(
                            ps,
                            wk_sb[:, kc, :],
                            kv_in[:, kc, :],
                            start=(kc == 0),
                            stop=(kc == D // 128 - 1),
                        )
                    sl = slice(j * CH, (j + 1) * CH)
                    nc.scalar.copy(out=kT_sb[:, sl], in_=ps)
                    # v: compute vT (feature-major, fast N) then PE-transpose
                    # into natural [tokens, feats] bf16 tiles
                    psvt = pa_ps.tile([FPC, CH], f32, tag="ps", name=f"psvt{j}")
                    for kc in range(D // 128):
                        nc.tensor.matmul(
                            psvt,
                            wv_sb[:, kc, :],
                            kv_in[:, kc, :],
                            start=(kc == 0),
                            stop=(kc == D // 128 - 1),
                        )
                    vt_t = pa_in.tile([FPC, CH], pdt, tag="vt_t", name=f"vt_t{j}")
                    nc.scalar.copy(out=vt_t, in_=psvt)
                    for s in range(CH // 128):
                        psv = pa_psv.tile([128, FPC], pdt, tag="psv", name=f"psv{j}_{s}")
                        nc.tensor.transpose(
                            psv,
                            vt_t[:, s * 128 : (s + 1) * 128],
                            ident_bf,
                        )
                        cidx = j * (CH // 128) + s
                        for hh in range(HPC):
                            nc.scalar.copy(
                                out=v_sb[:, cidx, hh, 0:DV],
                                in_=psv[:, hh * DV : (hh + 1) * DV],
                            )
                    r_in = pa_in.tile(
                        [128, D // 128, CH], pdt, tag="xin2", name=f"r_in{j}"
                    )
                    nc.scalar.dma_start(
                        out=r_in,
                        in_=xrT[:].rearrange("(kc p) t -> p kc t", p=128)[
                            :, :, j * CH : (j + 1) * CH
                        ],
                    )
                    ps2 = pa_ps.tile([FPC, CH], f32, tag="ps", name=f"ps_r{j}")
                    for kc in range(D // 128):
                        nc.tensor.matmul(
                            ps2,
                            wr_sb[:, kc, :],
                            r_in[:, kc, :],
                            start=(kc == 0),
                            stop=(kc == D // 128 - 1),
                        )
                    nc.scalar.copy(out=rT_sb[:, sl], in_=ps2)

                for j in range(2):
                    emit_q_chunk(j)
                for j in range(4):
                    emit_kvr_chunk(j)
                for j in range(2, 4):
                    emit_q_chunk(j)
                for j in range(4, 8):
                    emit_kvr_chunk(j)
                pa_psv.release()
                pa_ps.release()
                pa_in.release()

                # attention pools
                pb_rows = tc.alloc_tile_pool(name="pb_rows", bufs=2)
                pb_bdt = tc.alloc_tile_pool(name="pb_bdt", bufs=3)
                pb_sum = tc.alloc_tile_pool(name="pb_sum", bufs=3)
                pb_exp = tc.alloc_tile_pool(name="pb_exp", bufs=3)
                pb_bc = tc.alloc_tile_pool(name="pb_bc", bufs=2)
                pb_small = tc.alloc_tile_pool(name="pb_small", bufs=2)
                pb_ps = tc.alloc_tile_pool(name="pb_ps", bufs=2, space="PSUM")
                pb_ps2 = tc.alloc_tile_pool(name="pb_ps2", bufs=3, space="PSUM")
                pb_ctx = tc.alloc_tile_pool(name="pb_ctx", bufs=2, space="PSUM")
                pb_psb = tc.alloc_tile_pool(name="pb_psb", bufs=1, space="PSUM")

                def bd_raw(pi, b, hh, t):
                    # raw (unshifted) bd rows for q chunk t, q-major
                    qf = slice(64 * hh, 64 * hh + 64)
                    n0 = 1 if t < 4 else 0
                    bd_row = pb_rows.tile([128, TKV], bf16, tag="bd_row")
                    for n in range(n0, NK):
                        ps_bd = pb_ps.tile([128, 512], f32, tag="ps_sc")
                        nc.tensor.matmul(
                            ps_bd,
                            qpb_sb[qf, b * TQ + t * 128 : b * TQ + (t + 1) * 128],
                            rT_sb[qf, b * TKV + 512 * n : b * TKV + 512 * (n + 1)],
                            start=True,
                            stop=True,
                        )
                        if n % 2 == 0:
                            nc.scalar.copy(
                                out=bd_row[:, 512 * n : 512 * (n + 1)], in_=ps_bd
                            )
                        else:
                            nc.vector.tensor_copy(
                                out=bd_row[:, 512 * n : 512 * (n + 1)], in_=ps_bd
                            )
                    nc.scalar.dma_start(
                        out=AP(
                            tensor=bd_dram[pi][t // 4][:].tensor,
                            offset=(t % 4) * 128 * TKV + 512 * n0,
                            ap=[[TKV, 128], [1, TKV - 512 * n0]],
                        ),
                        in_=bd_row[:, 512 * n0 : TKV],
                    )

                def attn_half(pi, b, hh, h, fillers):
                    qf = slice(64 * hh, 64 * hh + 64)
                    kcmax = 12 + 4 * h
                    ps_ctx = pb_ctx.tile(
                        [DV + 1, 512], f32, tag="ps_ctx", name=f"psctx{pi}_{h}"
                    )
                    exp_tiles = {}

                    def score_stage(kc):
                        # shifted+transposed bd tile [kv 128, q 512] via XBAR
                        bdsT = pb_bdt.tile([128, 512], bf16, tag="bdsT")
                        nc.sync.dma_start(
                            out=bdsT,
                            in_=AP(
                                tensor=bd_dram[pi][h][:].tensor,
                                offset=(TQ - 1 - 512 * h) + 128 * kc,
                                ap=[[TKV - 1, 512], [1, 128]],
                            ),
                            transpose=True,
                        )
                        if kc >= 8 + 4 * h:
                            # keep where q >= k - R_OFF, i.e.
                            # j + (512h + R_OFF - 128 kc) - p >= 0
                            nc.gpsimd.affine_select(
                                out=bdsT,
                                in_=bdsT,
                                pattern=[[1, 512]],
                                compare_op=ALU.is_ge,
                                fill=-1e30,
                                base=512 * h + R_OFF - 128 * kc,
                                channel_multiplier=-1,
                            )
                        ps_sc = pb_ps2.tile([128, 512], f32, tag="ps_sc2")
                        nc.tensor.matmul(
                            ps_sc,
                            kT_sb[qf, b * TKV + 128 * kc : b * TKV + 128 * (kc + 1)],
                            qcb_sb[qf, b * TQ + 512 * h : b * TQ + 512 * (h + 1)],
                            start=True,
                            stop=True,
                        )
                        sum_sb = pb_sum.tile([128, 512], f32, tag="sum_sb")
                        nc.vector.tensor_add(out=sum_sb, in0=ps_sc, in1=bdsT)
                        expT = pb_exp.tile([128, 512], bf16, tag="expT")
                        nc.scalar.activation(
                            out=expT, in_=sum_sb, func=Exp, scale=0.125
                        )
                        exp_tiles[kc] = expT

                    def ctx_stage(kc):
                        nc.tensor.matmul(
                            ps_ctx,
                            v_sb[:, b * (TKV // 128) + kc, hh, :],
                            exp_tiles.pop(kc),
                            start=(kc == 0),
                            stop=(kc == kcmax - 1),
                        )

                    for kc in range(kcmax):
                        if fillers:
                            fillers.pop(0)()
                        score_stage(kc)
                        if kc >= 2:
                            ctx_stage(kc - 2)
                    ctx_stage(kcmax - 2)
                    ctx_stage(kcmax - 1)
                    # normalize: ctxT[f, q] *= 1/den[q], den in psum row DV
                    recip = pb_small.tile([1, 512], bf16, tag="recip")
                    with nc.allow_low_precision(
                        reason="bf16 1/denominator matches baseline attn bf16"
                    ):
                        nc.vector.reciprocal(recip, ps_ctx[DV : DV + 1, :])
                    ps_b = pb_psb.tile([DV, 512], f32, tag="ps_b")
                    nc.tensor.matmul(ps_b, ones_bf, recip, start=True, stop=True)
                    bcast = pb_bc.tile([DV, 512], f32, tag="bcast")
                    nc.scalar.copy(out=bcast, in_=ps_b)
                    nc.vector.tensor_mul(
                        out=ctx_sb[qf, b * TQ + 512 * h : b * TQ + 512 * (h + 1)],
                        in0=ps_ctx[0:DV, :],
                        in1=bcast,
                    )

                pairs = [(0, 0, 0), (1, 0, 1), (2, 1, 0), (3, 1, 1)]
                for idx, (pi, b, hh) in enumerate(pairs):
                    if idx == 0:
                        for t in range(4):
                            bd_raw(pi, b, hh, t)
                    import functools

                    fill0 = [
                        functools.partial(bd_raw, pi, b, hh, 4 + t) for t in range(4)
                    ]
                    attn_half(pi, b, hh, 0, fill0)
                    if idx + 1 < 4:
                        pj, bj, hj = pairs[idx + 1]
                        fill1 = [
                            functools.partial(bd_raw, pj, bj, hj, t) for t in range(4)
                        ]
                    else:
                        fill1 = []
                    attn_half(pi, b, hh, 1, fill1)

                pb_psb.release()
                pb_ctx.release()
                pb_ps2.release()
                pb_ps.release()
                pb_small.release()
                pb_bc.release()
                pb_exp.release()
                pb_sum.release()
                pb_bdt.release()
                pb_rows.release()

            # ================= Phase C: exchange + output projection + LN =========
            nc.sync.dma_start(
                out=a2a_in[:].rearrange("(j p) t -> p j t", p=FPC),
                in_=ctx_sb[:].rearrange("p (j t) -> p j t", t=RPC),
            )
            if n_cores > 1:
                nc.gpsimd.collective_compute(
                    "AllToAll",
                    ALU.bypass,
                    replica_groups=[list(range(n_cores))],
                    ins=[a2a_in[:]],
                    outs=[a2a_out[:]],
                )
            else:
                # single-core profiling variant: plain copy instead
                nc.sync.dma_start(out=a2a_out[:], in_=a2a_in[:])
            with tc.tile_pool(name="pc", bufs=3) as pc, tc.tile_pool(
                name="pc_ps", bufs=4, space="PSUM"
            ) as pc_ps, tc.tile_pool(name="pc_small", bufs=8) as pc_small:
                for mc in range(RPC // 128):
                    ps_o = [pc_ps.tile([128, 512], f32, tag="ps_o", name=f"ps_o{nn_}") for nn_ in range(2)]
                    for kc in range(D // 128):
                        lhs = pc.tile([128, 128], pdt, tag="octx")
                        nc.sync.dma_start(
                            out=lhs,
                            in_=a2a_out[
                                kc * 128 : (kc + 1) * 128, mc * 128 : (mc + 1) * 128
                            ],
                        )
                        for nn in range(2):
                            nc.tensor.matmul(
                                ps_o[nn],
                                lhs,
                                wo_sb[:, kc, nn * 512 : (nn + 1) * 512],
                                start=(kc == 0),
                                stop=(kc == D // 128 - 1),
                            )
                    o_sb = pc.tile([128, D], f32, tag="o_sb")
                    for nn in range(2):
                        nc.vector.tensor_add(
                            out=o_sb[:, nn * 512 : (nn + 1) * 512],
                            in0=ps_o[nn],
                            in1=qres_sb[:, mc, nn * 512 : (nn + 1) * 512],
                        )
                    # LayerNorm over the free (feature) dim
                    stats = pc_small.tile([128, 2, 6], f32, tag="stats")
                    for sg in range(2):
                        nc.vector.bn_stats(
                            out=stats[:, sg, :], in_=o_sb[:, sg * 512 : (sg + 1) * 512]
                        )
                    mv = pc_small.tile([128, 2], f32, tag="mv")
                    nc.vector.bn_aggr(out=mv, in_=stats)
                    mean, var = mv[:, 0:1], mv[:, 1:2]
                    xve = pc_small.tile([128, 1], f32, tag="xve")
                    nc.vector.tensor_scalar_add(out=xve, in0=var, scalar1=eps_sb)
                    std = pc_small.tile([128, 1], f32, tag="std")
                    nc.scalar.activation(out=std, in_=var, func=Sqrt, bias=eps_sb)
                    rstd = pc_small.tile([128, 1], f32, tag="rstd")
                    nc.vector.reciprocal(rstd, std)
                    # one Newton step for rsqrt accuracy:
                    # r <- r * (1.5 - 0.5 * x * r^2)
                    tnw = pc_small.tile([128, 1], f32, tag="tnw")
                    nc.vector.tensor_mul(out=tnw, in0=rstd, in1=rstd)
                    nc.vector.tensor_mul(out=tnw, in0=tnw, in1=xve)
                    nc.vector.tensor_scalar(
                        out=tnw, in0=tnw, scalar1=-0.5, scalar2=1.5,
                        op0=ALU.mult, op1=ALU.add,
                    )
                    nc.vector.tensor_scalar_mul(out=rstd, in0=rstd, scalar1=tnw)
                    nc.vector.tensor_scalar(
                        out=o_sb, in0=o_sb, scalar1=mean, scalar2=rstd,
                        op0=ALU.subtract, op1=ALU.mult,
                    )
                    nc.vector.tensor_mul(out=o_sb, in0=o_sb, in1=gamma_sb)
                    nc.vector.tensor_add(out=o_sb, in0=o_sb, in1=beta_sb)
                    nc.sync.dma_start(
                        out=out[mc * 128 : (mc + 1) * 128, :], in_=o_sb
                    )
    return nc


def _make_in_maps(inputs, mm_dtype="bfloat16"):
    query = np.asarray(inputs["query"], np.float32)
    key_value = np.asarray(inputs["key_value"], np.float32)
    relative = np.asarray(inputs["relative"], np.float32)
    content_bias = np.asarray(inputs["content_bias"], np.float32)
    position_bias = np.asarray(inputs["position_bias"], np.float32)
    Wq, Wk = np.asarray(inputs["Wq"], np.float32), np.asarray(inputs["Wk"], np.float32)
    Wv, Wr = np.asarray(inputs["Wv"], np.float32), np.asarray(inputs["Wr"], np.float32)
    Wo = np.ascontiguousarray(np.asarray(inputs["Wo"], np.float32))
    ln_gamma = np.asarray(inputs["ln_gamma"], np.float32)
    ln_beta = np.asarray(inputs["ln_beta"], np.float32)

    qflat = query.reshape(B * TQ, D)
    if mm_dtype == "bfloat16":
        import ml_dtypes

        mdt = ml_dtypes.bfloat16
    else:
        mdt = np.float32
    xqT = np.ascontiguousarray(qflat.T).astype(mdt)
    xkvT = np.ascontiguousarray(key_value.reshape(B * TKV, D).T).astype(mdt)
    xrT = np.ascontiguousarray(relative.reshape(B * TKV, D).T).astype(mdt)
    Wq, Wk = Wq.astype(mdt), Wk.astype(mdt)
    Wv, Wr = Wv.astype(mdt), Wr.astype(mdt)
    Wo = Wo.astype(mdt)
    cb = content_bias.reshape(NH, DV)
    pb = position_bias.reshape(NH, DV)

    in_maps = []
    for c in range(N_CORES):
        fs = slice(FPC * c, FPC * (c + 1))
        in_maps.append(
            {
                "xqT": xqT,
                "xkvT": xkvT,
                "xrT": xrT,
                "wq": np.ascontiguousarray(Wq[:, fs]),
                "wk": np.ascontiguousarray(Wk[:, fs]),
                "wv": np.ascontiguousarray(Wv[:, fs]),
                "wr": np.ascontiguousarray(Wr[:, fs]),
                "wo": Wo,
                "cbv": np.ascontiguousarray(
                    cb[HPC * c : HPC * (c + 1)].reshape(FPC, 1)
                ),
                "pbv": np.ascontiguousarray(
                    pb[HPC * c : HPC * (c + 1)].reshape(FPC, 1)
                ),
                "qres": np.ascontiguousarray(qflat[RPC * c : RPC * (c + 1)]),
                "gamma": ln_gamma,
                "beta": ln_beta,
            }
        )
    return in_maps


def run_on_hw(inputs, trace=False, score_dtype="bfloat16", proj_dtype="bfloat16"):
    from concourse.bass_utils import run_bass_kernel_spmd

    key = (score_dtype, proj_dtype)
    nc = _CACHE.get(key)
    if nc is None:
        nc = build_program(score_dtype=score_dtype, proj_dtype=proj_dtype)
        _CACHE[key] = nc
    in_maps = _make_in_maps(inputs, mm_dtype=proj_dtype)
    res = run_bass_kernel_spmd(nc, in_maps, list(range(N_CORES)), trace=trace)
    outs = np.concatenate(
        [np.asarray(res.results[c]["out"]) for c in range(N_CORES)], axis=0
    )
    return outs.reshape(B, TQ, D), res


def kernel(**inputs) -> np.ndarray:
    out, _ = run_on_hw(inputs)
    return out


# revision 16
# speedup vs baseline: 1.2074x; 1.0231x over previous
"""Trainium2 Bass kernel for Transformer-XL style relative-position attention.

Reference computation (B=2, Tq=1024, Tkv=2048, D=1024, H=16, Dv=64):
    q/k/v/r projections, ac = (q+cb)@k^T, bd = rel_shift((q+pb)@r^T),
    softmax((ac+bd)/8) with causal-with-memory mask, ctx = attn@v,
    out = LN(ctx@Wo + query).

Sharding (Megatron-style tensor parallel over heads, 8 cores):
  - each core owns 2 heads: Wq/Wk/Wv/Wr column shards [1024,128], biases
    shard with heads.
  - activations (transposed on host to feature-major) are broadcast.
  - after per-head attention, ctx^T shards are exchanged with a single
    AllToAll so that each core ends up with the full ctx^T for 1/8 of the
    token rows; each core then does that row-slice of ctx@Wo + residual +
    LayerNorm with the full Wo.

Device-side structure (transposed-scores design):
  - scores are computed TRANSPOSED (kv on partitions, q on the free dim):
    acT tiles come straight from a matmul with kT as the stationary
    operand; this makes the softmax output directly consumable by the
    attn@v contraction with NO attention-matrix transpose or DRAM
    round-trip.
  - rel_shift: bd is computed q-major (dense PE work), EXPONENTIATED
    (exp commutes with the shift), written bf16 to a flat DRAM scratch,
    and read back through a strided AP with row stride Tkv-1 PLUS
    transpose=True (hardware XBAR transpose), which lands the *shifted,
    transposed* exp(bd/8) tiles in SBUF in one step.
  - softmax numerator: exp((ac+bd)/8) = exp(ac/8) * exp(bd/8): the
    scalar engine exponentiates acT from PSUM, and the product runs on
    gpsimd/vector (SBUF-only operands), so no engine ever needs an
    (ac+bd) add against PSUM.
  - causal mask applied to the exp(bd) tiles with affine_select fill=0.
  - softmax denominators come for free from a ones-column appended to v:
    the attn@v matmul accumulates sum(exp) in psum row 64.
  - 1/denominator (per q) is broadcast across the 64 feature partitions
    with a rank-1 matmul (ones ⊗ recip) and applied to the small ctx^T
    tile instead of the big attention matrix.
"""

import numpy as np

# problem shapes (hardcoded per contract)
B, TQ, TKV, D, NH, DV = 2, 1024, 2048, 1024, 16, 64
N_CORES = 8
HPC = NH // N_CORES          # heads per core = 2
FPC = HPC * DV               # head-feature columns per core = 128
RPC = (B * TQ) // N_CORES    # output token rows per core = 256
R_OFF = TKV - TQ             # causal memory offset = 1024
LN_EPS = 1e-5
NT = TQ // 128               # query row chunks = 8
NK = TKV // 512              # key col chunks of 512 = 4

_CACHE = {}


def _patched_tc_class():
    """TileContext whose kernel-tail drain splits sem waits one per drain.

    The walrus build in this container rejects CTRL-type instructions
    (InstDrain) carrying more than one sync-wait command.
    """
    import concourse.mybir as mybir
    import concourse.tile as tile
    from concourse.vector_clock import ScopedClock

    class TC(tile.TileContext):
        def _commit_instruction(self, inst, lazy_reg_writes=True):
            # This walrus build rejects instructions carrying more than one
            # sync-wait command; hoist extras onto preceding NoOp carriers.
            si = getattr(inst, "sync_info", None)
            if (
                si is not None
                and si.on_wait
                and len(si.on_wait) > 1
                and inst.engine != mybir.EngineType.Unassigned
            ):
                waits = list(si.on_wait)
                inst.sync_info = mybir.SyncInfo(
                    on_wait=[waits[-1]], on_update=list(si.on_update or [])
                )
                for w in waits[:-1]:
                    ev = mybir.InstNoOp(
                        name=f"I-wsplit-{self.nc.next_id()}", ins=[], outs=[]
                    )
                    ev.engine = inst.engine
                    ev.sync_info = mybir.SyncInfo(on_wait=[w], on_update=[])
                    self._add_instruction(ev)
            return super()._commit_instruction(inst, lazy_reg_writes)

        def _drain_and_barrier(self, tick_clock, wait_clock):
            nc = self.nc
            drain_inst = nc.sync.drain()
            wait_clock.add_sem_waits(
                drain_inst.ins, ScopedClock({None: tick_clock.global_clock})
            )
            inner = drain_inst.ins
            si = inner.sync_info
            waits = list(si.on_wait) if si and si.on_wait else []
            if len(waits) > 1:
                inner.sync_info = mybir.SyncInfo(
                    on_wait=waits[:1], on_update=list(si.on_update or [])
                )
                for w in waits[1:]:
                    d2 = nc.sync.drain()
                    d2.ins.sync_info = mybir.SyncInfo(on_wait=[w], on_update=[])
            nc.all_engine_barrier()
            assert self.sems is not None
            popped = nc._tile_sem_poison_stack.pop()
            assert popped is self._sem_poison
            nc.clear_and_free_semaphores(list(self.sems.allocated().values()))
            nc.all_engine_barrier()

    return TC


def build_program(score_dtype="bfloat16", proj_dtype="bfloat16", n_cores=N_CORES):
    """Build the SPMD Bass program (identical on all 8 cores).

    n_cores=1 builds a single-core variant (collective replaced by a
    self-copy) for profiling; its output is only valid for core 0's
    feature shard.
    """
    import concourse.bass as bass
    import concourse.mybir as mybir
    from concourse.bass import AP

    f32 = mybir.dt.float32
    bf16 = mybir.dt.bfloat16
    pdt = bf16
    sdt = bf16
    TC = _patched_tc_class()

    nc = bass.Bass()

    # ---- I/O ----
    xqT = nc.dram_tensor("xqT", [D, B * TQ], pdt, kind="ExternalInput")
    xkvT = nc.dram_tensor("xkvT", [D, B * TKV], pdt, kind="ExternalInput")
    xrT = nc.dram_tensor("xrT", [D, B * TKV], pdt, kind="ExternalInput")
    wq = nc.dram_tensor("wq", [D, FPC], pdt, kind="ExternalInput")
    wk = nc.dram_tensor("wk", [D, FPC], pdt, kind="ExternalInput")
    wv = nc.dram_tensor("wv", [D, FPC], pdt, kind="ExternalInput")
    wr = nc.dram_tensor("wr", [D, FPC], pdt, kind="ExternalInput")
    wo = nc.dram_tensor("wo", [D, D], pdt, kind="ExternalInput")
    cbv = nc.dram_tensor("cbv", [FPC, 1], f32, kind="ExternalInput")
    pbv = nc.dram_tensor("pbv", [FPC, 1], f32, kind="ExternalInput")
    qres = nc.dram_tensor("qres", [RPC, D], f32, kind="ExternalInput")
    gamma = nc.dram_tensor("gamma", [D], f32, kind="ExternalInput")
    beta = nc.dram_tensor("beta", [D], f32, kind="ExternalInput")
    out = nc.dram_tensor("out", [RPC, D], f32, kind="ExternalOutput")

    # ---- internal DRAM scratch ----
    # raw bd per (pair, q-half): flat [512 rows x TKV]; the shifted,
    # transposed read only ever depends on its own half's rows.
    bd_dram = [
        [nc.dram_tensor(f"bd_dram{p}_{h}", [512 * TKV], bf16) for h in range(2)]
        for p in range(4)
    ]
    a2a_in = nc.dram_tensor("a2a_in", [N_CORES * FPC, RPC], pdt)
    a2a_out = nc.dram_tensor("a2a_out", [N_CORES * FPC, RPC], pdt)

    Exp = mybir.ActivationFunctionType.Exp
    Identity = mybir.ActivationFunctionType.Identity
    Sqrt = mybir.ActivationFunctionType.Sqrt
    ALU = mybir.AluOpType

    with TC(nc) as tc:
        import contextlib

        with contextlib.ExitStack() as ctx:
            singles = ctx.enter_context(tc.tile_pool(name="singles", bufs=1))

            # ---- static SBUF tensors ----
            wq_sb = singles.tile([128, D // 128, FPC], pdt, tag="wq_sb")
            wk_sb = singles.tile([128, D // 128, FPC], pdt, tag="wk_sb")
            wv_sb = singles.tile([128, D // 128, FPC], pdt, tag="wv_sb")
            wr_sb = singles.tile([128, D // 128, FPC], pdt, tag="wr_sb")
            for w_sb, w_dr in ((wq_sb, wq), (wk_sb, wk), (wv_sb, wv), (wr_sb, wr)):
                nc.gpsimd.dma_start(
                    out=w_sb, in_=w_dr[:].rearrange("(kc p) f -> p kc f", p=128)
                )
            wo_sb = singles.tile([128, D // 128, D], pdt, tag="wo_sb")
            nc.gpsimd.dma_start(
                out=wo_sb, in_=wo[:].rearrange("(kc p) d -> p kc d", p=128)
            )
            cb_sb = singles.tile([FPC, 1], f32, tag="cb_sb")
            pb_sb = singles.tile([FPC, 1], f32, tag="pb_sb")
            nc.sync.dma_start(out=cb_sb, in_=cbv[:])
            nc.sync.dma_start(out=pb_sb, in_=pbv[:])
            eps_sb = singles.tile([128, 1], f32, tag="eps_sb")
            nc.vector.memset(eps_sb, LN_EPS)
            gamma_sb = singles.tile([128, D], f32, tag="gamma_sb")
            beta_sb = singles.tile([128, D], f32, tag="beta_sb")
            nc.gpsimd.dma_start(
                out=gamma_sb,
                in_=AP(tensor=gamma[:].tensor, offset=0, ap=[[0, 128], [1, D]]),
            )
            nc.gpsimd.dma_start(
                out=beta_sb,
                in_=AP(tensor=beta[:].tensor, offset=0, ap=[[0, 128], [1, D]]),
            )
            qres_sb = singles.tile([128, RPC // 128, D], f32, tag="qres_sb")
            nc.gpsimd.dma_start(
                out=qres_sb, in_=qres[:].rearrange("(mc p) d -> p mc d", p=128)
            )

            # projection outputs (feature-major, both heads stacked on partitions)
            qcb_sb = singles.tile([FPC, B * TQ], sdt, tag="qcb_sb")
            qpb_sb = singles.tile([FPC, B * TQ], sdt, tag="qpb_sb")
            kT_sb = singles.tile([FPC, B * TKV], sdt, tag="kT_sb")
            rT_sb = singles.tile([FPC, B * TKV], sdt, tag="rT_sb")
            # v in natural layout [kv-token partitions, chunk, head, 64+ones]
            v_sb = singles.tile([128, (B * TKV) // 128, HPC, DV + 1], bf16, tag="v_sb")
            nc.vector.memset(v_sb[:, :, :, DV], 1.0)
            ctx_sb = singles.tile([FPC, B * TQ], pdt, tag="ctx_sb")
            ones_bf = singles.tile([1, DV], bf16, tag="ones_bf")
            nc.vector.memset(ones_bf, 1.0)

            # identity (bf16) for PE-transposes
            ident_bf = singles.tile([128, 128], bf16, tag="ident_bf")
            from concourse.masks import make_identity

            make_identity(nc, ident_bf)

            # ========== Phases A+B interleaved: projections + attention ==========
            CH = 512  # token columns per projection step
            with contextlib.ExitStack() as phase_ab:
                pa_in = tc.alloc_tile_pool(name="pa_in", bufs=2)
                pa_ps = tc.alloc_tile_pool(name="pa_ps", bufs=4, space="PSUM")
                pa_psv = tc.alloc_tile_pool(name="pa_psv", bufs=4, space="PSUM")

                def emit_q_chunk(j):
                    q_in = pa_in.tile(
                        [128, D // 128, CH], pdt, tag="xin", name=f"q_in{j}"
                    )
                    nc.sync.dma_start(
                        out=q_in,
                        in_=xqT[:].rearrange("(kc p) t -> p kc t", p=128)[
                            :, :, j * CH : (j + 1) * CH
                        ],
                    )
                    ps = pa_ps.tile([FPC, CH], f32, tag="ps", name=f"ps_q{j}")
                    for kc in range(D // 128):
                        nc.tensor.matmul(
                            ps,
                            wq_sb[:, kc, :],
                            q_in[:, kc, :],
                            start=(kc == 0),
                            stop=(kc == D // 128 - 1),
                        )
                    sl = slice(j * CH, (j + 1) * CH)
                    nc.vector.tensor_scalar_add(
                        out=qcb_sb[:, sl], in0=ps, scalar1=cb_sb
                    )
                    nc.vector.tensor_scalar_add(
                        out=qpb_sb[:, sl], in0=ps, scalar1=pb_sb
                    )

                def emit_kvr_chunk(j):
                    kv_in = pa_in.tile(
                        [128, D // 128, CH], pdt, tag="xin", name=f"kv_in{j}"
                    )
                    nc.sync.dma_start(
                        out=kv_in,
                        in_=xkvT[:].rearrange("(kc p) t -> p kc t", p=128)[
                            :, :, j * CH : (j + 1) * CH
                        ],
                    )
                    ps = pa_ps.tile([FPC, CH], f32, tag="ps", name=f"ps_k{j}")
                    for kc in range(D // 128):
                        nc.tensor.matmul(
                            ps,
                            wk_sb[:, kc, :],
                            kv_in[:, kc, :],
                            start=(kc == 0),
                            stop=(kc == D // 128 - 1),
                        )
                    sl = slice(j * CH, (j + 1) * CH)
                    nc.vector.tensor_copy(out=kT_sb[:, sl], in_=ps)
                    # v: compute vT (feature-major, fast N) then PE-transpose
                    # into natural [tokens, feats] bf16 tiles
                    psvt = pa_ps.tile([FPC, CH], f32, tag="ps", name=f"psvt{j}")
                    for kc in range(D // 128):
                        nc.tensor.matmul(
                            psvt,
                            wv_sb[:, kc, :],
                            kv_in[:, kc, :],
                            start=(kc == 0),
                            stop=(kc == D // 128 - 1),
                        )
                    vt_t = pa_in.tile([FPC, CH], pdt, tag="vt_t", name=f"vt_t{j}")
                    nc.vector.tensor_copy(out=vt_t, in_=psvt)
                    for s in range(CH // 128):
                        psv = pa_psv.tile([128, FPC], pdt, tag="psv", name=f"psv{j}_{s}")
                        nc.tensor.transpose(
                            psv,
                            vt_t[:, s * 128 : (s + 1) * 128],
                            ident_bf,
                        )
                        cidx = j * (CH // 128) + s
                        for hh in range(HPC):
                            nc.scalar.copy(
                                out=v_sb[:, cidx, hh, 0:DV],
                                in_=psv[:, hh * DV : (hh + 1) * DV],
                            )
                    r_in = pa_in.tile(
                        [128, D // 128, CH], pdt, tag="xin2", name=f"r_in{j}"
                    )
                    nc.gpsimd.dma_start(
                        out=r_in,
                        in_=xrT[:].rearrange("(kc p) t -> p kc t", p=128)[
                            :, :, j * CH : (j + 1) * CH
                        ],
                    )
                    ps2 = pa_ps.tile([FPC, CH], f32, tag="ps", name=f"ps_r{j}")
                    for kc in range(D // 128):
                        nc.tensor.matmul(
                            ps2,
                            wr_sb[:, kc, :],
                            r_in[:, kc, :],
                            start=(kc == 0),
                            stop=(kc == D // 128 - 1),
                        )
                    nc.vector.tensor_copy(out=rT_sb[:, sl], in_=ps2)

                for j in range(2):
                    emit_q_chunk(j)
                for j in range(4):
                    emit_kvr_chunk(j)
                for j in range(2, 4):
                    emit_q_chunk(j)
                for j in range(4, 8):
                    emit_kvr_chunk(j)
                pa_psv.release()
                pa_ps.release()
                pa_in.release()

                # attention pools
                pb_rows = tc.alloc_tile_pool(name="pb_rows", bufs=2)
                pb_bdt = tc.alloc_tile_pool(name="pb_bdt", bufs=6)
                pb_eac = tc.alloc_tile_pool(name="pb_eac", bufs=4)
                pb_prod = tc.alloc_tile_pool(name="pb_prod", bufs=4)
                pb_bc = tc.alloc_tile_pool(name="pb_bc", bufs=2)
                pb_small = tc.alloc_tile_pool(name="pb_small", bufs=2)
                pb_ps = tc.alloc_tile_pool(name="pb_ps", bufs=2, space="PSUM")
                pb_ps2 = tc.alloc_tile_pool(name="pb_ps2", bufs=3, space="PSUM")
                pb_ctx = tc.alloc_tile_pool(name="pb_ctx", bufs=2, space="PSUM")
                pb_psb = tc.alloc_tile_pool(name="pb_psb", bufs=1, space="PSUM")

                def bd_raw(pi, b, hh, t):
                    # exp(bd/8) rows for q chunk t, q-major, unshifted
                    qf = slice(64 * hh, 64 * hh + 64)
                    n0 = 1 if t < 4 else 0
                    bd_row = pb_rows.tile([128, TKV], bf16, tag="bd_row")
                    for n in range(n0, NK):
                        ps_bd = pb_ps.tile([128, 512], f32, tag="ps_sc")
                        nc.tensor.matmul(
                            ps_bd,
                            qpb_sb[qf, b * TQ + t * 128 : b * TQ + (t + 1) * 128],
                            rT_sb[qf, b * TKV + 512 * n : b * TKV + 512 * (n + 1)],
                            start=True,
                            stop=True,
                        )
                        nc.scalar.activation(
                            out=bd_row[:, 512 * n : 512 * (n + 1)],
                            in_=ps_bd,
                            func=Exp,
                            scale=0.125,
                        )
                    nc.gpsimd.dma_start(
                        out=AP(
                            tensor=bd_dram[pi][t // 4][:].tensor,
                            offset=(t % 4) * 128 * TKV + 512 * n0,
                            ap=[[TKV, 128], [1, TKV - 512 * n0]],
                        ),
                        in_=bd_row[:, 512 * n0 : TKV],
                    )

                def attn_half(pi, b, hh, h, fillers):
                    qf = slice(64 * hh, 64 * hh + 64)
                    kcmax = 12 + 4 * h
                    PF = 4  # transposed-read prefetch depth
                    ps_ctx = pb_ctx.tile(
                        [DV + 1, 512], f32, tag="ps_ctx", name=f"psctx{pi}_{h}"
                    )
                    bd_tiles = {}
                    prod_tiles = {}

                    def issue_read(kc):
                        # shifted+transposed exp(bd) tile [kv 128, q 512];
                        # only rows covering the unmasked q range are read.
                        qlo = max(0, 128 * (kc - 8) - 512 * h)
                        ebdT = pb_bdt.tile([128, 512], bf16, tag="ebdT")
                        nc.sync.dma_start(
                            out=ebdT[:, qlo:512],
                            in_=AP(
                                tensor=bd_dram[pi][h][:].tensor,
                                offset=(TQ - 1 - 512 * h)
                                + 128 * kc
                                + qlo * (TKV - 1),
                                ap=[[TKV - 1, 512 - qlo], [1, 128]],
                            ),
                            transpose=True,
                        )
                        bd_tiles[kc] = ebdT

                    def score_stage(kc):
                        eng = nc.gpsimd if kc % 2 == 0 else nc.vector
                        if kc >= 8 + 4 * h:
                            # keep where q >= k - R_OFF, i.e.
                            # j + (512h + R_OFF - 128 kc) - p >= 0
                            nc.gpsimd.affine_select(
                                out=bd_tiles[kc],
                                in_=bd_tiles[kc],
                                pattern=[[1, 512]],
                                compare_op=ALU.is_ge,
                                fill=0.0,
                                base=512 * h + R_OFF - 128 * kc,
                                channel_multiplier=-1,
                            )
                        ps_sc = pb_ps2.tile([128, 512], f32, tag="ps_sc2")
                        nc.tensor.matmul(
                            ps_sc,
                            kT_sb[qf, b * TKV + 128 * kc : b * TKV + 128 * (kc + 1)],
                            qcb_sb[qf, b * TQ + 512 * h : b * TQ + 512 * (h + 1)],
                            start=True,
                            stop=True,
                        )
                        eacT = pb_eac.tile([128, 512], bf16, tag="eacT")
                        nc.scalar.activation(
                            out=eacT, in_=ps_sc, func=Exp, scale=0.125
                        )
                        expT = pb_prod.tile([128, 512], bf16, tag="expT")
                        eng.tensor_mul(out=expT, in0=eacT, in1=bd_tiles.pop(kc))
                        prod_tiles[kc] = expT

                    def ctx_stage(kc):
                        nc.tensor.matmul(
                            ps_ctx,
                            v_sb[:, b * (TKV // 128) + kc, hh, :],
                            prod_tiles.pop(kc),
                            start=(kc == 0),
                            stop=(kc == kcmax - 1),
                        )

                    for k2 in range(min(PF, kcmax)):
                        issue_read(k2)
                    for kc in range(kcmax):
                        if fillers:
                            fillers.pop(0)()
                        if kc + PF < kcmax:
                            issue_read(kc + PF)
                        score_stage(kc)
                        if kc >= 2:
                            ctx_stage(kc - 2)
                    ctx_stage(kcmax - 2)
                    ctx_stage(kcmax - 1)
                    # normalize: ctxT[f, q] *= 1/den[q], den in psum row DV
                    recip = pb_small.tile([1, 512], bf16, tag="recip")
                    with nc.allow_low_precision(
                        reason="bf16 1/denominator matches baseline attn bf16"
                    ):
                        nc.vector.reciprocal(recip, ps_ctx[DV : DV + 1, :])
                    ps_b = pb_psb.tile([DV, 512], f32, tag="ps_b")
                    nc.tensor.matmul(ps_b, ones_bf, recip, start=True, stop=True)
                    bcast = pb_bc.tile([DV, 512], f32, tag="bcast")
                    nc.scalar.copy(out=bcast, in_=ps_b)
                    nc.vector.tensor_mul(
                        out=ctx_sb[qf, b * TQ + 512 * h : b * TQ + 512 * (h + 1)],
                        in0=ps_ctx[0:DV, :],
                        in1=bcast,
                    )

                pairs = [(0, 0, 0), (1, 0, 1), (2, 1, 0), (3, 1, 1)]
                for idx, (pi, b, hh) in enumerate(pairs):
                    if idx == 0:
                        for t in range(4):
                            bd_raw(pi, b, hh, t)
                    import functools

                    fill0 = [
                        functools.partial(bd_raw, pi, b, hh, 4 + t) for t in range(4)
                    ]
                    attn_half(pi, b, hh, 0, fill0)
                    if idx + 1 < 4:
                        pj, bj, hj = pairs[idx + 1]
                        fill1 = [
                            functools.partial(bd_raw, pj, bj, hj, t) for t in range(4)
                        ]
                    else:
                        fill1 = []
                    attn_half(pi, b, hh, 1, fill1)

                pb_psb.release()
                pb_ctx.release()
                pb_ps2.release()
                pb_ps.release()
                pb_small.release()
                pb_bc.release()
                pb_prod.release()
                pb_eac.release()
                pb_bdt.release()
                pb_rows.release()

            # ================= Phase C: exchange + output projection + LN =========
            nc.sync.dma_start(
                out=a2a_in[:].rearrange("(j p) t -> p j t", p=FPC),
                in_=ctx_sb[:].rearrange("p (j t) -> p j t", t=RPC),
            )
            if n_cores > 1:
                nc.gpsimd.collective_compute(
                    "AllToAll",
                    ALU.bypass,
                    replica_groups=[list(range(n_cores))],
                    ins=[a2a_in[:]],
                    outs=[a2a_out[:]],
                )
            else:
                # single-core profiling variant: plain copy instead
                nc.sync.dma_start(out=a2a_out[:], in_=a2a_in[:])
            with tc.tile_pool(name="pc", bufs=3) as pc, tc.tile_pool(
                name="pc_ps", bufs=4, space="PSUM"
            ) as pc_ps, tc.tile_pool(name="pc_small", bufs=8) as pc_small:
                for mc in range(RPC // 128):
                    ps_o = [pc_ps.tile([128, 512], f32, tag="ps_o", name=f"ps_o{nn_}") for nn_ in range(2)]
                    for kc in range(D // 128):
                        lhs = pc.tile([128, 128], pdt, tag="octx")
                        nc.sync.dma_start(
                            out=lhs,
                            in_=a2a_out[
                                kc * 128 : (kc + 1) * 128, mc * 128 : (mc + 1) * 128
                            ],
                        )
                        for nn in range(2):
                            nc.tensor.matmul(
                                ps_o[nn],
                                lhs,
                                wo_sb[:, kc, nn * 512 : (nn + 1) * 512],
                                start=(kc == 0),
                                stop=(kc == D // 128 - 1),
                            )
                    o_sb = pc.tile([128, D], f32, tag="o_sb")
                    for nn in range(2):
                        nc.vector.tensor_add(
                            out=o_sb[:, nn * 512 : (nn + 1) * 512],
                            in0=ps_o[nn],
                            in1=qres_sb[:, mc, nn * 512 : (nn + 1) * 512],
                        )
                    # LayerNorm over the free (feature) dim
                    stats = pc_small.tile([128, 2, 6], f32, tag="stats")
                    for sg in range(2):
                        nc.vector.bn_stats(
                            out=stats[:, sg, :], in_=o_sb[:, sg * 512 : (sg + 1) * 512]
                        )
                    mv = pc_small.tile([128, 2], f32, tag="mv")
                    nc.vector.bn_aggr(out=mv, in_=stats)
                    mean, var = mv[:, 0:1], mv[:, 1:2]
                    xve = pc_small.tile([128, 1], f32, tag="xve")
                    nc.vector.tensor_scalar_add(out=xve, in0=var, scalar1=eps_sb)
                    std = pc_small.tile([128, 1], f32, tag="std")
                    nc.scalar.activation(out=std, in_=var, func=Sqrt, bias=eps_sb)
                    rstd = pc_small.tile([128, 1], f32, tag="rstd")
                    nc.vector.reciprocal(rstd, std)
                    # one Newton step for rsqrt accuracy:
                    # r <- r * (1.5 - 0.5 * x * r^2)
                    tnw = pc_small.tile([128, 1], f32, tag="tnw")
                    nc.vector.tensor_mul(out=tnw, in0=rstd, in1=rstd)
                    nc.vector.tensor_mul(out=tnw, in0=tnw, in1=xve)
                    nc.vector.tensor_scalar(
                        out=tnw, in0=tnw, scalar1=-0.5, scalar2=1.5,
                        op0=ALU.mult, op1=ALU.add,
                    )
                    nc.vector.tensor_scalar_mul(out=rstd, in0=rstd, scalar1=tnw)
                    nc.vector.tensor_scalar(
                        out=o_sb, in0=o_sb, scalar1=mean, scalar2=rstd,
                        op0=ALU.subtract, op1=ALU.mult,
                    )
                    nc.vector.tensor_mul(out=o_sb, in0=o_sb, in1=gamma_sb)
                    nc.vector.tensor_add(out=o_sb, in0=o_sb, in1=beta_sb)
                    nc.sync.dma_start(
                        out=out[mc * 128 : (mc + 1) * 128, :], in_=o_sb
                    )
    return nc


def _make_in_maps(inputs, mm_dtype="bfloat16"):
    query = np.asarray(inputs["query"], np.float32)
    key_value = np.asarray(inputs["key_value"], np.float32)
    relative = np.asarray(inputs["relative"], np.float32)
    content_bias = np.asarray(inputs["content_bias"], np.float32)
    position_bias = np.asarray(inputs["position_bias"], np.float32)
    Wq, Wk = np.asarray(inputs["Wq"], np.float32), np.asarray(inputs["Wk"], np.float32)
    Wv, Wr = np.asarray(inputs["Wv"], np.float32), np.asarray(inputs["Wr"], np.float32)
    Wo = np.ascontiguousarray(np.asarray(inputs["Wo"], np.float32))
    ln_gamma = np.asarray(inputs["ln_gamma"], np.float32)
    ln_beta = np.asarray(inputs["ln_beta"], np.float32)

    qflat = query.reshape(B * TQ, D)
    if mm_dtype == "bfloat16":
        import ml_dtypes

        mdt = ml_dtypes.bfloat16
    else:
        mdt = np.float32
    xqT = np.ascontiguousarray(qflat.T).astype(mdt)
    xkvT = np.ascontiguousarray(key_value.reshape(B * TKV, D).T).astype(mdt)
    xrT = np.ascontiguousarray(relative.reshape(B * TKV, D).T).astype(mdt)
    Wq, Wk = Wq.astype(mdt), Wk.astype(mdt)
    Wv, Wr = Wv.astype(mdt), Wr.astype(mdt)
    Wo = Wo.astype(mdt)
    cb = content_bias.reshape(NH, DV)
    pb = position_bias.reshape(NH, DV)

    in_maps = []
    for c in range(N_CORES):
        fs = slice(FPC * c, FPC * (c + 1))
        in_maps.append(
            {
                "xqT": xqT,
                "xkvT": xkvT,
                "xrT": xrT,
                "wq": np.ascontiguousarray(Wq[:, fs]),
                "wk": np.ascontiguousarray(Wk[:, fs]),
                "wv": np.ascontiguousarray(Wv[:, fs]),
                "wr": np.ascontiguousarray(Wr[:, fs]),
                "wo": Wo,
                "cbv": np.ascontiguousarray(
                    cb[HPC * c : HPC * (c + 1)].reshape(FPC, 1)
                ),
                "pbv": np.ascontiguousarray(
                    pb[HPC * c : HPC * (c + 1)].reshape(FPC, 1)
                ),
                "qres": np.ascontiguousarray(qflat[RPC * c : RPC * (c + 1)]),
                "gamma": ln_gamma,
                "beta": ln_beta,
            }
        )
    return in_maps


def run_on_hw(inputs, trace=False, score_dtype="bfloat16", proj_dtype="bfloat16"):
    from concourse.bass_utils import run_bass_kernel_spmd

    key = (score_dtype, proj_dtype)
    nc = _CACHE.get(key)
    if nc is None:
        nc = build_program(score_dtype=score_dtype, proj_dtype=proj_dtype)
        _CACHE[key] = nc
    in_maps = _make_in_maps(inputs, mm_dtype=proj_dtype)
    res = run_bass_kernel_spmd(nc, in_maps, list(range(N_CORES)), trace=trace)
    outs = np.concatenate(
        [np.asarray(res.results[c]["out"]) for c in range(N_CORES)], axis=0
    )
    return outs.reshape(B, TQ, D), res


def kernel(**inputs) -> np.ndarray:
    out, _ = run_on_hw(inputs)
    return out


# revision 21
# speedup vs baseline: 1.2816x; 1.0614x over previous
"""Trainium2 Bass kernel for Transformer-XL style relative-position attention.

Reference computation (B=2, Tq=1024, Tkv=2048, D=1024, H=16, Dv=64):
    q/k/v/r projections, ac = (q+cb)@k^T, bd = rel_shift((q+pb)@r^T),
    softmax((ac+bd)/8) with causal-with-memory mask, ctx = attn@v,
    out = LN(ctx@Wo + query).

Sharding (Megatron-style tensor parallel over heads, 8 cores):
  - each core owns 2 heads: Wq/Wk/Wv/Wr column shards [1024,128], biases
    shard with heads.
  - activations (transposed on host to feature-major) are broadcast.
  - after per-head attention, ctx^T shards are exchanged with a single
    AllToAll so that each core ends up with the full ctx^T for 1/8 of the
    token rows; each core then does that row-slice of ctx@Wo + residual +
    LayerNorm with the full Wo.

Device-side structure (transposed-scores design):
  - scores are computed TRANSPOSED (kv on partitions, q on the free dim):
    acT tiles come straight from a matmul with kT as the stationary
    operand; this makes the softmax output directly consumable by the
    attn@v contraction with NO attention-matrix transpose or DRAM
    round-trip.
  - rel_shift: bd is computed q-major (dense PE work), EXPONENTIATED
    (exp commutes with the shift), written bf16 to a flat DRAM scratch,
    and read back through a strided AP with row stride Tkv-1 PLUS
    transpose=True (hardware XBAR transpose), which lands the *shifted,
    transposed* exp(bd/8) tiles in SBUF in one step.
  - softmax numerator: exp((ac+bd)/8) = exp(ac/8) * exp(bd/8): the
    scalar engine exponentiates acT from PSUM, and the product runs on
    gpsimd/vector (SBUF-only operands), so no engine ever needs an
    (ac+bd) add against PSUM.
  - causal mask applied to the exp(bd) tiles with affine_select fill=0.
  - softmax denominators come for free from a ones-column appended to v:
    the attn@v matmul accumulates sum(exp) in psum row 64.
  - 1/denominator (per q) is broadcast across the 64 feature partitions
    with a rank-1 matmul (ones ⊗ recip) and applied to the small ctx^T
    tile instead of the big attention matrix.
"""

import numpy as np

# problem shapes (hardcoded per contract)
B, TQ, TKV, D, NH, DV = 2, 1024, 2048, 1024, 16, 64
N_CORES = 8
HPC = NH // N_CORES          # heads per core = 2
FPC = HPC * DV               # head-feature columns per core = 128
RPC = (B * TQ) // N_CORES    # output token rows per core = 256
R_OFF = TKV - TQ             # causal memory offset = 1024
LN_EPS = 1e-5
NT = TQ // 128               # query row chunks = 8
NK = TKV // 512              # key col chunks of 512 = 4

_CACHE = {}


def _patched_tc_class():
    """TileContext whose kernel-tail drain splits sem waits one per drain.

    The walrus build in this container rejects CTRL-type instructions
    (InstDrain) carrying more than one sync-wait command.
    """
    import concourse.mybir as mybir
    import concourse.tile as tile
    from concourse.vector_clock import ScopedClock

    class TC(tile.TileContext):
        def _commit_instruction(self, inst, lazy_reg_writes=True):
            # This walrus build rejects instructions carrying more than one
            # sync-wait command; hoist extras onto preceding NoOp carriers.
            si = getattr(inst, "sync_info", None)
            if (
                si is not None
                and si.on_wait
                and len(si.on_wait) > 1
                and inst.engine != mybir.EngineType.Unassigned
            ):
                waits = list(si.on_wait)
                inst.sync_info = mybir.SyncInfo(
                    on_wait=[waits[-1]], on_update=list(si.on_update or [])
                )
                for w in waits[:-1]:
                    ev = mybir.InstNoOp(
                        name=f"I-wsplit-{self.nc.next_id()}", ins=[], outs=[]
                    )
                    ev.engine = inst.engine
                    ev.sync_info = mybir.SyncInfo(on_wait=[w], on_update=[])
                    self._add_instruction(ev)
            return super()._commit_instruction(inst, lazy_reg_writes)

        def _drain_and_barrier(self, tick_clock, wait_clock):
            nc = self.nc
            drain_inst = nc.sync.drain()
            wait_clock.add_sem_waits(
                drain_inst.ins, ScopedClock({None: tick_clock.global_clock})
            )
            inner = drain_inst.ins
            si = inner.sync_info
            waits = list(si.on_wait) if si and si.on_wait else []
            if len(waits) > 1:
                inner.sync_info = mybir.SyncInfo(
                    on_wait=waits[:1], on_update=list(si.on_update or [])
                )
                for w in waits[1:]:
                    d2 = nc.sync.drain()
                    d2.ins.sync_info = mybir.SyncInfo(on_wait=[w], on_update=[])
            nc.all_engine_barrier()
            assert self.sems is not None
            popped = nc._tile_sem_poison_stack.pop()
            assert popped is self._sem_poison
            nc.clear_and_free_semaphores(list(self.sems.allocated().values()))
            nc.all_engine_barrier()

    return TC


def build_program(score_dtype="bfloat16", proj_dtype="bfloat16", n_cores=N_CORES):
    """Build the SPMD Bass program (identical on all 8 cores).

    n_cores=1 builds a single-core variant (collective replaced by a
    self-copy) for profiling; its output is only valid for core 0's
    feature shard.
    """
    import concourse.bass as bass
    import concourse.mybir as mybir
    from concourse.bass import AP

    f32 = mybir.dt.float32
    bf16 = mybir.dt.bfloat16
    pdt = bf16
    sdt = bf16
    TC = _patched_tc_class()

    nc = bass.Bass()

    # ---- I/O ----
    xqT = nc.dram_tensor("xqT", [D, B * TQ], pdt, kind="ExternalInput")
    xkvT = nc.dram_tensor("xkvT", [D, B * TKV], pdt, kind="ExternalInput")
    xrT = nc.dram_tensor("xrT", [D, B * TKV], pdt, kind="ExternalInput")
    wq = nc.dram_tensor("wq", [D, FPC], pdt, kind="ExternalInput")
    wk = nc.dram_tensor("wk", [D, FPC], pdt, kind="ExternalInput")
    wv = nc.dram_tensor("wv", [D, FPC], pdt, kind="ExternalInput")
    wr = nc.dram_tensor("wr", [D, FPC], pdt, kind="ExternalInput")
    wo = nc.dram_tensor("wo", [D, D], pdt, kind="ExternalInput")
    cbv = nc.dram_tensor("cbv", [FPC, 1], f32, kind="ExternalInput")
    pbv = nc.dram_tensor("pbv", [FPC, 1], f32, kind="ExternalInput")
    qres = nc.dram_tensor("qres", [RPC, D], f32, kind="ExternalInput")
    gamma = nc.dram_tensor("gamma", [D], f32, kind="ExternalInput")
    beta = nc.dram_tensor("beta", [D], f32, kind="ExternalInput")
    out = nc.dram_tensor("out", [RPC, D], f32, kind="ExternalOutput")

    # ---- internal DRAM scratch ----
    # raw bd per (pair, q-half): flat [512 rows x TKV]; the shifted,
    # transposed read only ever depends on its own half's rows.
    bd_dram = [
        [nc.dram_tensor(f"bd_dram{p}_{h}", [512 * TKV], bf16) for h in range(2)]
        for p in range(4)
    ]
    a2a_in = nc.dram_tensor("a2a_in", [N_CORES * FPC, RPC], pdt)
    a2a_out = nc.dram_tensor("a2a_out", [N_CORES * FPC, RPC], pdt)

    Exp = mybir.ActivationFunctionType.Exp
    Identity = mybir.ActivationFunctionType.Identity
    Sqrt = mybir.ActivationFunctionType.Sqrt
    ALU = mybir.AluOpType

    with TC(nc) as tc:
        import contextlib

        with contextlib.ExitStack() as ctx:
            singles = ctx.enter_context(tc.tile_pool(name="singles", bufs=1))

            # ---- static SBUF tensors ----
            wq_sb = singles.tile([128, D // 128, FPC], pdt, tag="wq_sb")
            wk_sb = singles.tile([128, D // 128, FPC], pdt, tag="wk_sb")
            wv_sb = singles.tile([128, D // 128, FPC], pdt, tag="wv_sb")
            wr_sb = singles.tile([128, D // 128, FPC], pdt, tag="wr_sb")
            for w_sb, w_dr in ((wq_sb, wq), (wk_sb, wk), (wv_sb, wv), (wr_sb, wr)):
                nc.gpsimd.dma_start(
                    out=w_sb, in_=w_dr[:].rearrange("(kc p) f -> p kc f", p=128)
                )
            wo_sb = singles.tile([128, D // 128, D], pdt, tag="wo_sb")
            nc.gpsimd.dma_start(
                out=wo_sb, in_=wo[:].rearrange("(kc p) d -> p kc d", p=128)
            )
            cb_sb = singles.tile([FPC, 1], f32, tag="cb_sb")
            pb_sb = singles.tile([FPC, 1], f32, tag="pb_sb")
            nc.sync.dma_start(out=cb_sb, in_=cbv[:])
            nc.sync.dma_start(out=pb_sb, in_=pbv[:])
            eps_sb = singles.tile([128, 1], f32, tag="eps_sb")
            nc.vector.memset(eps_sb, LN_EPS)
            gamma_sb = singles.tile([128, D], f32, tag="gamma_sb")
            beta_sb = singles.tile([128, D], f32, tag="beta_sb")
            nc.gpsimd.dma_start(
                out=gamma_sb,
                in_=AP(tensor=gamma[:].tensor, offset=0, ap=[[0, 128], [1, D]]),
            )
            nc.gpsimd.dma_start(
                out=beta_sb,
                in_=AP(tensor=beta[:].tensor, offset=0, ap=[[0, 128], [1, D]]),
            )
            qres_sb = singles.tile([128, RPC // 128, D], f32, tag="qres_sb")
            nc.gpsimd.dma_start(
                out=qres_sb, in_=qres[:].rearrange("(mc p) d -> p mc d", p=128)
            )

            # projection outputs (feature-major, both heads stacked on partitions)
            qcb_sb = singles.tile([FPC, B * TQ], sdt, tag="qcb_sb")
            qpb_sb = singles.tile([FPC, B * TQ], sdt, tag="qpb_sb")
            kT_sb = singles.tile([FPC, B * TKV], sdt, tag="kT_sb")
            rT_sb = singles.tile([FPC, B * TKV], sdt, tag="rT_sb")
            # v in natural layout [kv-token partitions, chunk, head, 64+ones]
            v_sb = singles.tile([128, (B * TKV) // 128, HPC, DV + 1], bf16, tag="v_sb")
            nc.vector.memset(v_sb[:, :, :, DV], 1.0)
            ctx_sb = singles.tile([FPC, B * TQ], pdt, tag="ctx_sb")
            ones_bf = singles.tile([1, DV], bf16, tag="ones_bf")
            nc.vector.memset(ones_bf, 1.0)

            # identity (bf16) for PE-transposes
            ident_bf = singles.tile([128, 128], bf16, tag="ident_bf")
            from concourse.masks import make_identity

            make_identity(nc, ident_bf)

            # ========== Phases A+B interleaved: projections + attention ==========
            CH = 512  # token columns per projection step
            with contextlib.ExitStack() as phase_ab:
                pa_in = tc.alloc_tile_pool(name="pa_in", bufs=2)
                pa_ps = tc.alloc_tile_pool(name="pa_ps", bufs=4, space="PSUM")
                pa_psv = tc.alloc_tile_pool(name="pa_psv", bufs=4, space="PSUM")

                def emit_q_chunk(j):
                    q_in = pa_in.tile(
                        [128, D // 128, CH], pdt, tag="xin", name=f"q_in{j}"
                    )
                    nc.sync.dma_start(
                        out=q_in,
                        in_=xqT[:].rearrange("(kc p) t -> p kc t", p=128)[
                            :, :, j * CH : (j + 1) * CH
                        ],
                    )
                    ps = pa_ps.tile([FPC, CH], f32, tag="ps", name=f"ps_q{j}")
                    for kc in range(D // 128):
                        nc.tensor.matmul(
                            ps,
                            wq_sb[:, kc, :],
                            q_in[:, kc, :],
                            start=(kc == 0),
                            stop=(kc == D // 128 - 1),
                        )
                    sl = slice(j * CH, (j + 1) * CH)
                    nc.vector.tensor_scalar_add(
                        out=qcb_sb[:, sl], in0=ps, scalar1=cb_sb
                    )
                    nc.vector.tensor_scalar_add(
                        out=qpb_sb[:, sl], in0=ps, scalar1=pb_sb
                    )

                def emit_kvr_chunk(j):
                    kv_in = pa_in.tile(
                        [128, D // 128, CH], pdt, tag="xin", name=f"kv_in{j}"
                    )
                    nc.sync.dma_start(
                        out=kv_in,
                        in_=xkvT[:].rearrange("(kc p) t -> p kc t", p=128)[
                            :, :, j * CH : (j + 1) * CH
                        ],
                    )
                    ps = pa_ps.tile([FPC, CH], f32, tag="ps", name=f"ps_k{j}")
                    for kc in range(D // 128):
                        nc.tensor.matmul(
                            ps,
                            wk_sb[:, kc, :],
                            kv_in[:, kc, :],
                            start=(kc == 0),
                            stop=(kc == D // 128 - 1),
                        )
                    sl = slice(j * CH, (j + 1) * CH)
                    nc.vector.tensor_copy(out=kT_sb[:, sl], in_=ps)
                    # v: compute vT (feature-major, fast N) then PE-transpose
                    # into natural [tokens, feats] bf16 tiles
                    psvt = pa_ps.tile([FPC, CH], f32, tag="ps", name=f"psvt{j}")
                    for kc in range(D // 128):
                        nc.tensor.matmul(
                            psvt,
                            wv_sb[:, kc, :],
                            kv_in[:, kc, :],
                            start=(kc == 0),
                            stop=(kc == D // 128 - 1),
                        )
                    vt_t = pa_in.tile([FPC, CH], pdt, tag="vt_t", name=f"vt_t{j}")
                    nc.vector.tensor_copy(out=vt_t, in_=psvt)
                    for s in range(CH // 128):
                        psv = pa_psv.tile([128, FPC], pdt, tag="psv", name=f"psv{j}_{s}")
                        nc.tensor.transpose(
                            psv,
                            vt_t[:, s * 128 : (s + 1) * 128],
                            ident_bf,
                        )
                        cidx = j * (CH // 128) + s
                        for hh in range(HPC):
                            nc.scalar.copy(
                                out=v_sb[:, cidx, hh, 0:DV],
                                in_=psv[:, hh * DV : (hh + 1) * DV],
                            )
                    r_in = pa_in.tile(
                        [128, D // 128, CH], pdt, tag="xin2", name=f"r_in{j}"
                    )
                    nc.scalar.dma_start(
                        out=r_in,
                        in_=xrT[:].rearrange("(kc p) t -> p kc t", p=128)[
                            :, :, j * CH : (j + 1) * CH
                        ],
                    )
                    ps2 = pa_ps.tile([FPC, CH], f32, tag="ps", name=f"ps_r{j}")
                    for kc in range(D // 128):
                        nc.tensor.matmul(
                            ps2,
                            wr_sb[:, kc, :],
                            r_in[:, kc, :],
                            start=(kc == 0),
                            stop=(kc == D // 128 - 1),
                        )
                    nc.vector.tensor_copy(out=rT_sb[:, sl], in_=ps2)

                for j in range(2):
                    emit_q_chunk(j)
                for j in range(4):
                    emit_kvr_chunk(j)
                for j in range(2, 4):
                    emit_q_chunk(j)
                for j in range(4, 8):
                    emit_kvr_chunk(j)
                pa_psv.release()
                pa_ps.release()
                pa_in.release()

                # attention pools
                pb_rows = tc.alloc_tile_pool(name="pb_rows", bufs=2)
                pb_bdt = tc.alloc_tile_pool(name="pb_bdt", bufs=6)
                pb_eac = tc.alloc_tile_pool(name="pb_eac", bufs=4)
                pb_prod = tc.alloc_tile_pool(name="pb_prod", bufs=4)
                pb_bc = tc.alloc_tile_pool(name="pb_bc", bufs=2)
                pb_small = tc.alloc_tile_pool(name="pb_small", bufs=2)
                pb_ps = tc.alloc_tile_pool(name="pb_ps", bufs=2, space="PSUM")
                pb_ps2 = tc.alloc_tile_pool(name="pb_ps2", bufs=3, space="PSUM")
                pb_ctx = tc.alloc_tile_pool(name="pb_ctx", bufs=2, space="PSUM")
                pb_psb = tc.alloc_tile_pool(name="pb_psb", bufs=1, space="PSUM")

                def bd_raw_stages(pi, b, hh, t):
                    # exp(bd/8) rows for q chunk t, q-major, unshifted.
                    # Returned as fine-grained thunks (one matmul+exp each,
                    # plus the DMA write) so they interleave with the score
                    # pipeline without bunching up on the scalar engine.
                    qf = slice(64 * hh, 64 * hh + 64)
                    n0 = 1 if t < 4 else 0
                    state = {}

                    def mk_mm(n):
                        def thunk():
                            if "row" not in state:
                                state["row"] = pb_rows.tile(
                                    [128, TKV],
                                    bf16,
                                    tag="bd_row",
                                    name=f"bd_row{pi}_{t}",
                                )
                            ps_bd = pb_ps.tile([128, 512], f32, tag="ps_sc")
                            nc.tensor.matmul(
                                ps_bd,
                                qpb_sb[
                                    qf, b * TQ + t * 128 : b * TQ + (t + 1) * 128
                                ],
                                rT_sb[
                                    qf, b * TKV + 512 * n : b * TKV + 512 * (n + 1)
                                ],
                                start=True,
                                stop=True,
                            )
                            nc.scalar.activation(
                                out=state["row"][:, 512 * n : 512 * (n + 1)],
                                in_=ps_bd,
                                func=Exp,
                                scale=0.125,
                            )

                        return thunk

                    def wr_thunk():
                        nc.gpsimd.dma_start(
                            out=AP(
                                tensor=bd_dram[pi][t // 4][:].tensor,
                                offset=(t % 4) * 128 * TKV + 512 * n0,
                                ap=[[TKV, 128], [1, TKV - 512 * n0]],
                            ),
                            in_=state["row"][:, 512 * n0 : TKV],
                        )

                    return [mk_mm(n) for n in range(n0, NK)] + [wr_thunk]

                def attn_half(pi, b, hh, h, fillers, pending_finish):
                    qf = slice(64 * hh, 64 * hh + 64)
                    kcmax = 12 + 4 * h
                    PF = 4  # transposed-read prefetch depth
                    ps_ctx = pb_ctx.tile(
                        [DV + 1, 512], f32, tag="ps_ctx", name=f"psctx{pi}_{h}"
                    )
                    bd_tiles = {}
                    prod_tiles = {}

                    def issue_read(kc):
                        # shifted+transposed exp(bd) tile [kv 128, q 512];
                        # only rows covering the unmasked q range are read.
                        qlo = max(0, 128 * (kc - 8) - 512 * h)
                        ebdT = pb_bdt.tile([128, 512], bf16, tag="ebdT")
                        nc.sync.dma_start(
                            out=ebdT[:, qlo:512],
                            in_=AP(
                                tensor=bd_dram[pi][h][:].tensor,
                                offset=(TQ - 1 - 512 * h)
                                + 128 * kc
                                + qlo * (TKV - 1),
                                ap=[[TKV - 1, 512 - qlo], [1, 128]],
                            ),
                            transpose=True,
                        )
                        bd_tiles[kc] = ebdT

                    def score_stage(kc):
                        eng = nc.vector
                        if kc >= 8 + 4 * h:
                            # keep where q >= k - R_OFF, i.e.
                            # j + (512h + R_OFF - 128 kc) - p >= 0
                            nc.gpsimd.affine_select(
                                out=bd_tiles[kc],
                                in_=bd_tiles[kc],
                                pattern=[[1, 512]],
                                compare_op=ALU.is_ge,
                                fill=0.0,
                                base=512 * h + R_OFF - 128 * kc,
                                channel_multiplier=-1,
                            )
                        ps_sc = pb_ps2.tile([128, 512], f32, tag="ps_sc2")
                        nc.tensor.matmul(
                            ps_sc,
                            kT_sb[qf, b * TKV + 128 * kc : b * TKV + 128 * (kc + 1)],
                            qcb_sb[qf, b * TQ + 512 * h : b * TQ + 512 * (h + 1)],
                            start=True,
                            stop=True,
                        )
                        eacT = pb_eac.tile([128, 512], bf16, tag="eacT")
                        nc.scalar.activation(
                            out=eacT, in_=ps_sc, func=Exp, scale=0.125
                        )
                        expT = pb_prod.tile([128, 512], bf16, tag="expT")
                        eng.tensor_mul(out=expT, in0=eacT, in1=bd_tiles.pop(kc))
                        prod_tiles[kc] = expT

                    def ctx_stage(kc):
                        nc.tensor.matmul(
                            ps_ctx,
                            v_sb[:, b * (TKV // 128) + kc, hh, :],
                            prod_tiles.pop(kc),
                            start=(kc == 0),
                            stop=(kc == kcmax - 1),
                        )

                    for k2 in range(min(PF, kcmax)):
                        issue_read(k2)
                    for kc in range(kcmax):
                        if kc + PF < kcmax:
                            issue_read(kc + PF)
                        score_stage(kc)
                        if kc >= 2:
                            ctx_stage(kc - 2)
                        if kc == 1 and pending_finish is not None:
                            pending_finish()
                            pending_finish = None
                        # spread filler stages so the scalar engine never
                        # queues bd-exp work ahead of the critical eacT exp
                        slots_left = kcmax - kc
                        take = (len(fillers) + slots_left - 1) // slots_left
                        for _ in range(min(take, len(fillers))):
                            fillers.pop(0)()
                    ctx_stage(kcmax - 2)
                    ctx_stage(kcmax - 1)
                    if pending_finish is not None:
                        pending_finish()

                    def finish():
                        # normalize: ctxT[f, q] *= 1/den[q], den in psum row DV
                        recip = pb_small.tile([1, 512], bf16, tag="recip")
                        with nc.allow_low_precision(
                            reason="bf16 1/denominator matches baseline attn bf16"
                        ):
                            nc.vector.reciprocal(recip, ps_ctx[DV : DV + 1, :])
                        ps_b = pb_psb.tile([DV, 512], f32, tag="ps_b")
                        nc.tensor.matmul(
                            ps_b, ones_bf, recip, start=True, stop=True
                        )
                        bcast = pb_bc.tile([DV, 512], f32, tag="bcast")
                        nc.scalar.copy(out=bcast, in_=ps_b)
                        nc.vector.tensor_mul(
                            out=ctx_sb[
                                qf, b * TQ + 512 * h : b * TQ + 512 * (h + 1)
                            ],
                            in0=ps_ctx[0:DV, :],
                            in1=bcast,
                        )

                    return finish

                pairs = [(0, 0, 0), (1, 0, 1), (2, 1, 0), (3, 1, 1)]
                pending = None
                for idx, (pi, b, hh) in enumerate(pairs):
                    if idx == 0:
                        for t in range(4):
                            for th in bd_raw_stages(pi, b, hh, t):
                                th()
                    fill0 = []
                    for t in range(4):
                        fill0 += bd_raw_stages(pi, b, hh, 4 + t)
                    pending = attn_half(pi, b, hh, 0, fill0, pending)
                    fill1 = []
                    if idx + 1 < 4:
                        pj, bj, hj = pairs[idx + 1]
                        for t in range(4):
                            fill1 += bd_raw_stages(pj, bj, hj, t)
                    pending = attn_half(pi, b, hh, 1, fill1, pending)
                if pending is not None:
                    pending()

                pb_psb.release()
                pb_ctx.release()
                pb_ps2.release()
                pb_ps.release()
                pb_small.release()
                pb_bc.release()
                pb_prod.release()
                pb_eac.release()
                pb_bdt.release()
                pb_rows.release()

            # ================= Phase C: exchange + output projection + LN =========
            nc.sync.dma_start(
                out=a2a_in[:].rearrange("(j p) t -> p j t", p=FPC),
                in_=ctx_sb[:].rearrange("p (j t) -> p j t", t=RPC),
            )
            if n_cores > 1:
                nc.gpsimd.collective_compute(
                    "AllToAll",
                    ALU.bypass,
                    replica_groups=[list(range(n_cores))],
                    ins=[a2a_in[:]],
                    outs=[a2a_out[:]],
                )
            else:
                # single-core profiling variant: plain copy instead
                nc.sync.dma_start(out=a2a_out[:], in_=a2a_in[:])
            with tc.tile_pool(name="pc", bufs=3) as pc, tc.tile_pool(
                name="pc_ps", bufs=4, space="PSUM"
            ) as pc_ps, tc.tile_pool(name="pc_small", bufs=8) as pc_small:
                for mc in range(RPC // 128):
                    ps_o = [pc_ps.tile([128, 512], f32, tag="ps_o", name=f"ps_o{nn_}") for nn_ in range(2)]
                    for kc in range(D // 128):
                        lhs = pc.tile([128, 128], pdt, tag="octx")
                        nc.sync.dma_start(
                            out=lhs,
                            in_=a2a_out[
                                kc * 128 : (kc + 1) * 128, mc * 128 : (mc + 1) * 128
                            ],
                        )
                        for nn in range(2):
                            nc.tensor.matmul(
                                ps_o[nn],
                                lhs,
                                wo_sb[:, kc, nn * 512 : (nn + 1) * 512],
                                start=(kc == 0),
                                stop=(kc == D // 128 - 1),
                            )
                    o_sb = pc.tile([128, D], f32, tag="o_sb")
                    for nn in range(2):
                        nc.vector.tensor_add(
                            out=o_sb[:, nn * 512 : (nn + 1) * 512],
                            in0=ps_o[nn],
                            in1=qres_sb[:, mc, nn * 512 : (nn + 1) * 512],
                        )
                    # LayerNorm over the free (feature) dim
                    stats = pc_small.tile([128, 2, 6], f32, tag="stats")
                    for sg in range(2):
                        nc.vector.bn_stats(
                            out=stats[:, sg, :], in_=o_sb[:, sg * 512 : (sg + 1) * 512]
                        )
                    mv = pc_small.tile([128, 2], f32, tag="mv")
                    nc.vector.bn_aggr(out=mv, in_=stats)
                    mean, var = mv[:, 0:1], mv[:, 1:2]
                    xve = pc_small.tile([128, 1], f32, tag="xve")
                    nc.vector.tensor_scalar_add(out=xve, in0=var, scalar1=eps_sb)
                    std = pc_small.tile([128, 1], f32, tag="std")
                    nc.scalar.activation(out=std, in_=var, func=Sqrt, bias=eps_sb)
                    rstd = pc_small.tile([128, 1], f32, tag="rstd")
                    nc.vector.reciprocal(rstd, std)
                    # one Newton step for rsqrt accuracy:
                    # r <- r * (1.5 - 0.5 * x * r^2)
                    tnw = pc_small.tile([128, 1], f32, tag="tnw")
                    nc.vector.tensor_mul(out=tnw, in0=rstd, in1=rstd)
                    nc.vector.tensor_mul(out=tnw, in0=tnw, in1=xve)
                    nc.vector.tensor_scalar(
                        out=tnw, in0=tnw, scalar1=-0.5, scalar2=1.5,
                        op0=ALU.mult, op1=ALU.add,
                    )
                    nc.vector.tensor_scalar_mul(out=rstd, in0=rstd, scalar1=tnw)
                    nc.vector.tensor_scalar(
                        out=o_sb, in0=o_sb, scalar1=mean, scalar2=rstd,
                        op0=ALU.subtract, op1=ALU.mult,
                    )
                    nc.vector.tensor_mul(out=o_sb, in0=o_sb, in1=gamma_sb)
                    nc.vector.tensor_add(out=o_sb, in0=o_sb, in1=beta_sb)
                    nc.sync.dma_start(
                        out=out[mc * 128 : (mc + 1) * 128, :], in_=o_sb
                    )
    return nc


def _make_in_maps(inputs, mm_dtype="bfloat16"):
    query = np.asarray(inputs["query"], np.float32)
    key_value = np.asarray(inputs["key_value"], np.float32)
    relative = np.asarray(inputs["relative"], np.float32)
    content_bias = np.asarray(inputs["content_bias"], np.float32)
    position_bias = np.asarray(inputs["position_bias"], np.float32)
    Wq, Wk = np.asarray(inputs["Wq"], np.float32), np.asarray(inputs["Wk"], np.float32)
    Wv, Wr = np.asarray(inputs["Wv"], np.float32), np.asarray(inputs["Wr"], np.float32)
    Wo = np.ascontiguousarray(np.asarray(inputs["Wo"], np.float32))
    ln_gamma = np.asarray(inputs["ln_gamma"], np.float32)
    ln_beta = np.asarray(inputs["ln_beta"], np.float32)

    qflat = query.reshape(B * TQ, D)
    if mm_dtype == "bfloat16":
        import ml_dtypes

        mdt = ml_dtypes.bfloat16
    else:
        mdt = np.float32
    xqT = np.ascontiguousarray(qflat.T).astype(mdt)
    xkvT = np.ascontiguousarray(key_value.reshape(B * TKV, D).T).astype(mdt)
    xrT = np.ascontiguousarray(relative.reshape(B * TKV, D).T).astype(mdt)
    Wq, Wk = Wq.astype(mdt), Wk.astype(mdt)
    Wv, Wr = Wv.astype(mdt), Wr.astype(mdt)
    Wo = Wo.astype(mdt)
    cb = content_bias.reshape(NH, DV)
    pb = position_bias.reshape(NH, DV)

    in_maps = []
    for c in range(N_CORES):
        fs = slice(FPC * c, FPC * (c + 1))
        in_maps.append(
            {
                "xqT": xqT,
                "xkvT": xkvT,
                "xrT": xrT,
                "wq": np.ascontiguousarray(Wq[:, fs]),
                "wk": np.ascontiguousarray(Wk[:, fs]),
                "wv": np.ascontiguousarray(Wv[:, fs]),
                "wr": np.ascontiguousarray(Wr[:, fs]),
                "wo": Wo,
                "cbv": np.ascontiguousarray(
                    cb[HPC * c : HPC * (c + 1)].reshape(FPC, 1)
                ),
                "pbv": np.ascontiguousarray(
                    pb[HPC * c : HPC * (c + 1)].reshape(FPC, 1)
                ),
                "qres": np.ascontiguousarray(qflat[RPC * c : RPC * (c + 1)]),
                "gamma": ln_gamma,
                "beta": ln_beta,
            }
        )
    return in_maps


def run_on_hw(inputs, trace=False, score_dtype="bfloat16", proj_dtype="bfloat16"):
    from concourse.bass_utils import run_bass_kernel_spmd

    key = (score_dtype, proj_dtype)
    nc = _CACHE.get(key)
    if nc is None:
        nc = build_program(score_dtype=score_dtype, proj_dtype=proj_dtype)
        _CACHE[key] = nc
    in_maps = _make_in_maps(inputs, mm_dtype=proj_dtype)
    res = run_bass_kernel_spmd(nc, in_maps, list(range(N_CORES)), trace=trace)
    outs = np.concatenate(
        [np.asarray(res.results[c]["out"]) for c in range(N_CORES)], axis=0
    )
    return outs.reshape(B, TQ, D), res


def kernel(**inputs) -> np.ndarray:
    out, _ = run_on_hw(inputs)
    return out


# revision 23
# speedup vs baseline: 1.3573x; 1.0591x over previous
"""Trainium2 Bass kernel for Transformer-XL style relative-position attention.

Reference computation (B=2, Tq=1024, Tkv=2048, D=1024, H=16, Dv=64):
    q/k/v/r projections, ac = (q+cb)@k^T, bd = rel_shift((q+pb)@r^T),
    softmax((ac+bd)/8) with causal-with-memory mask, ctx = attn@v,
    out = LN(ctx@Wo + query).

Sharding (Megatron-style tensor parallel over heads, 8 cores):
  - each core owns 2 heads: Wq/Wk/Wv/Wr column shards [1024,128], biases
    shard with heads.
  - activations (transposed on host to feature-major) are broadcast.
  - after per-head attention, ctx^T shards are exchanged with a single
    AllToAll so that each core ends up with the full ctx^T for 1/8 of the
    token rows; each core then does that row-slice of ctx@Wo + residual +
    LayerNorm with the full Wo.

Device-side structure (transposed-scores design):
  - scores are computed TRANSPOSED (kv on partitions, q on the free dim):
    acT tiles come straight from a matmul with kT as the stationary
    operand; this makes the softmax output directly consumable by the
    attn@v contraction with NO attention-matrix transpose or DRAM
    round-trip.
  - rel_shift: bd is computed q-major (dense PE work), EXPONENTIATED
    (exp commutes with the shift), written bf16 to a flat DRAM scratch,
    and read back through a strided AP with row stride Tkv-1 PLUS
    transpose=True (hardware XBAR transpose), which lands the *shifted,
    transposed* exp(bd/8) tiles in SBUF in one step.
  - softmax numerator: exp((ac+bd)/8) = exp(ac/8) * exp(bd/8): the
    scalar engine exponentiates acT from PSUM, and the product runs on
    gpsimd/vector (SBUF-only operands), so no engine ever needs an
    (ac+bd) add against PSUM.
  - causal mask applied to the exp(bd) tiles with affine_select fill=0.
  - softmax denominators come for free from a ones-column appended to v:
    the attn@v matmul accumulates sum(exp) in psum row 64.
  - 1/denominator (per q) is broadcast across the 64 feature partitions
    with a rank-1 matmul (ones ⊗ recip) and applied to the small ctx^T
    tile instead of the big attention matrix.
"""

import numpy as np

# problem shapes (hardcoded per contract)
B, TQ, TKV, D, NH, DV = 2, 1024, 2048, 1024, 16, 64
N_CORES = 8
HPC = NH // N_CORES          # heads per core = 2
FPC = HPC * DV               # head-feature columns per core = 128
RPC = (B * TQ) // N_CORES    # output token rows per core = 256
R_OFF = TKV - TQ             # causal memory offset = 1024
LN_EPS = 1e-5
NT = TQ // 128               # query row chunks = 8
NK = TKV // 512              # key col chunks of 512 = 4

_CACHE = {}


def _patched_tc_class():
    """TileContext whose kernel-tail drain splits sem waits one per drain.

    The walrus build in this container rejects CTRL-type instructions
    (InstDrain) carrying more than one sync-wait command.
    """
    import concourse.mybir as mybir
    import concourse.tile as tile
    from concourse.vector_clock import ScopedClock

    class TC(tile.TileContext):
        def _commit_instruction(self, inst, lazy_reg_writes=True):
            # This walrus build rejects instructions carrying more than one
            # sync-wait command; hoist extras onto preceding NoOp carriers.
            si = getattr(inst, "sync_info", None)
            if (
                si is not None
                and si.on_wait
                and len(si.on_wait) > 1
                and inst.engine != mybir.EngineType.Unassigned
            ):
                waits = list(si.on_wait)
                inst.sync_info = mybir.SyncInfo(
                    on_wait=[waits[-1]], on_update=list(si.on_update or [])
                )
                for w in waits[:-1]:
                    ev = mybir.InstNoOp(
                        name=f"I-wsplit-{self.nc.next_id()}", ins=[], outs=[]
                    )
                    ev.engine = inst.engine
                    ev.sync_info = mybir.SyncInfo(on_wait=[w], on_update=[])
                    self._add_instruction(ev)
            return super()._commit_instruction(inst, lazy_reg_writes)

        def _drain_and_barrier(self, tick_clock, wait_clock):
            nc = self.nc
            drain_inst = nc.sync.drain()
            wait_clock.add_sem_waits(
                drain_inst.ins, ScopedClock({None: tick_clock.global_clock})
            )
            inner = drain_inst.ins
            si = inner.sync_info
            waits = list(si.on_wait) if si and si.on_wait else []
            if len(waits) > 1:
                inner.sync_info = mybir.SyncInfo(
                    on_wait=waits[:1], on_update=list(si.on_update or [])
                )
                for w in waits[1:]:
                    d2 = nc.sync.drain()
                    d2.ins.sync_info = mybir.SyncInfo(on_wait=[w], on_update=[])
            nc.all_engine_barrier()
            assert self.sems is not None
            popped = nc._tile_sem_poison_stack.pop()
            assert popped is self._sem_poison
            nc.clear_and_free_semaphores(list(self.sems.allocated().values()))
            nc.all_engine_barrier()

    return TC


def build_program(score_dtype="bfloat16", proj_dtype="bfloat16", n_cores=N_CORES):
    """Build the SPMD Bass program (identical on all 8 cores).

    n_cores=1 builds a single-core variant (collective replaced by a
    self-copy) for profiling; its output is only valid for core 0's
    feature shard.
    """
    import concourse.bass as bass
    import concourse.mybir as mybir
    from concourse.bass import AP

    f32 = mybir.dt.float32
    bf16 = mybir.dt.bfloat16
    pdt = bf16
    sdt = bf16
    TC = _patched_tc_class()

    nc = bass.Bass()

    # ---- I/O ----
    xqT = nc.dram_tensor("xqT", [D, B * TQ], pdt, kind="ExternalInput")
    xkvT = nc.dram_tensor("xkvT", [D, B * TKV], pdt, kind="ExternalInput")
    xrT = nc.dram_tensor("xrT", [D, B * TKV], pdt, kind="ExternalInput")
    wq = nc.dram_tensor("wq", [D, FPC], pdt, kind="ExternalInput")
    wk = nc.dram_tensor("wk", [D, FPC], pdt, kind="ExternalInput")
    wv = nc.dram_tensor("wv", [D, FPC], pdt, kind="ExternalInput")
    wr = nc.dram_tensor("wr", [D, FPC], pdt, kind="ExternalInput")
    wo = nc.dram_tensor("wo", [D, D], pdt, kind="ExternalInput")
    cbv = nc.dram_tensor("cbv", [FPC, 1], f32, kind="ExternalInput")
    pbv = nc.dram_tensor("pbv", [FPC, 1], f32, kind="ExternalInput")
    qres = nc.dram_tensor("qres", [RPC, D], f32, kind="ExternalInput")
    gamma = nc.dram_tensor("gamma", [D], f32, kind="ExternalInput")
    beta = nc.dram_tensor("beta", [D], f32, kind="ExternalInput")
    out = nc.dram_tensor("out", [RPC, D], f32, kind="ExternalOutput")

    # ---- internal DRAM scratch ----
    # raw bd per (pair, q-half): flat [512 rows x TKV]; the shifted,
    # transposed read only ever depends on its own half's rows.
    bd_dram = [
        [nc.dram_tensor(f"bd_dram{p}_{h}", [512 * TKV], bf16) for h in range(2)]
        for p in range(4)
    ]
    a2a_in = nc.dram_tensor("a2a_in", [N_CORES * FPC, RPC], pdt)
    a2a_out = nc.dram_tensor("a2a_out", [N_CORES * FPC, RPC], pdt)

    Exp = mybir.ActivationFunctionType.Exp
    Identity = mybir.ActivationFunctionType.Identity
    Sqrt = mybir.ActivationFunctionType.Sqrt
    ALU = mybir.AluOpType

    with TC(nc) as tc:
        import contextlib

        with contextlib.ExitStack() as ctx:
            singles = ctx.enter_context(tc.tile_pool(name="singles", bufs=1))

            # ---- static SBUF tensors ----
            wq_sb = singles.tile([128, D // 128, FPC], pdt, tag="wq_sb")
            wk_sb = singles.tile([128, D // 128, FPC], pdt, tag="wk_sb")
            wv_sb = singles.tile([128, D // 128, FPC], pdt, tag="wv_sb")
            wr_sb = singles.tile([128, D // 128, FPC], pdt, tag="wr_sb")
            for w_sb, w_dr in ((wq_sb, wq), (wk_sb, wk), (wv_sb, wv), (wr_sb, wr)):
                nc.gpsimd.dma_start(
                    out=w_sb, in_=w_dr[:].rearrange("(kc p) f -> p kc f", p=128)
                )
            wo_sb = singles.tile([128, D // 128, D], pdt, tag="wo_sb")
            nc.gpsimd.dma_start(
                out=wo_sb, in_=wo[:].rearrange("(kc p) d -> p kc d", p=128)
            )
            cb_sb = singles.tile([FPC, 1], f32, tag="cb_sb")
            pb_sb = singles.tile([FPC, 1], f32, tag="pb_sb")
            nc.sync.dma_start(out=cb_sb, in_=cbv[:])
            nc.sync.dma_start(out=pb_sb, in_=pbv[:])
            eps_sb = singles.tile([128, 1], f32, tag="eps_sb")
            nc.vector.memset(eps_sb, LN_EPS)
            gamma_sb = singles.tile([128, D], f32, tag="gamma_sb")
            beta_sb = singles.tile([128, D], f32, tag="beta_sb")
            nc.gpsimd.dma_start(
                out=gamma_sb,
                in_=AP(tensor=gamma[:].tensor, offset=0, ap=[[0, 128], [1, D]]),
            )
            nc.gpsimd.dma_start(
                out=beta_sb,
                in_=AP(tensor=beta[:].tensor, offset=0, ap=[[0, 128], [1, D]]),
            )
            qres_sb = singles.tile([128, RPC // 128, D], f32, tag="qres_sb")
            nc.gpsimd.dma_start(
                out=qres_sb, in_=qres[:].rearrange("(mc p) d -> p mc d", p=128)
            )

            # projection outputs (feature-major, both heads stacked on partitions)
            qcb_sb = singles.tile([FPC, B * TQ], sdt, tag="qcb_sb")
            qpb_sb = singles.tile([FPC, B * TQ], sdt, tag="qpb_sb")
            kT_sb = singles.tile([FPC, B * TKV], sdt, tag="kT_sb")
            rT_sb = singles.tile([FPC, B * TKV], sdt, tag="rT_sb")
            # v in natural layout [kv-token partitions, chunk, head, 64+ones]
            v_sb = singles.tile([128, (B * TKV) // 128, HPC, DV + 1], bf16, tag="v_sb")
            nc.vector.memset(v_sb[:, :, :, DV], 1.0)
            ctx_sb = singles.tile([FPC, B * TQ], pdt, tag="ctx_sb")
            ones_bf = singles.tile([1, DV], bf16, tag="ones_bf")
            nc.vector.memset(ones_bf, 1.0)

            # identity (bf16) for PE-transposes
            ident_bf = singles.tile([128, 128], bf16, tag="ident_bf")
            from concourse.masks import make_identity

            make_identity(nc, ident_bf)

            # ========== Phases A+B interleaved: projections + attention ==========
            CH = 512  # token columns per projection step
            with contextlib.ExitStack() as phase_ab:
                pa_in = tc.alloc_tile_pool(name="pa_in", bufs=2)
                pa_ps = tc.alloc_tile_pool(name="pa_ps", bufs=4, space="PSUM")
                pa_psv = tc.alloc_tile_pool(name="pa_psv", bufs=4, space="PSUM")

                def emit_q_chunk(j):
                    q_in = pa_in.tile(
                        [128, D // 128, CH], pdt, tag="xin", name=f"q_in{j}"
                    )
                    nc.sync.dma_start(
                        out=q_in,
                        in_=xqT[:].rearrange("(kc p) t -> p kc t", p=128)[
                            :, :, j * CH : (j + 1) * CH
                        ],
                    )
                    ps = pa_ps.tile([FPC, CH], f32, tag="ps", name=f"ps_q{j}")
                    for kc in range(D // 128):
                        nc.tensor.matmul(
                            ps,
                            wq_sb[:, kc, :],
                            q_in[:, kc, :],
                            start=(kc == 0),
                            stop=(kc == D // 128 - 1),
                        )
                    sl = slice(j * CH, (j + 1) * CH)
                    nc.vector.tensor_scalar_add(
                        out=qcb_sb[:, sl], in0=ps, scalar1=cb_sb
                    )
                    nc.vector.tensor_scalar_add(
                        out=qpb_sb[:, sl], in0=ps, scalar1=pb_sb
                    )

                def emit_kvr_chunk(j):
                    kv_in = pa_in.tile(
                        [128, D // 128, CH], pdt, tag="xin", name=f"kv_in{j}"
                    )
                    nc.sync.dma_start(
                        out=kv_in,
                        in_=xkvT[:].rearrange("(kc p) t -> p kc t", p=128)[
                            :, :, j * CH : (j + 1) * CH
                        ],
                    )
                    ps = pa_ps.tile([FPC, CH], f32, tag="ps", name=f"ps_k{j}")
                    for kc in range(D // 128):
                        nc.tensor.matmul(
                            ps,
                            wk_sb[:, kc, :],
                            kv_in[:, kc, :],
                            start=(kc == 0),
                            stop=(kc == D // 128 - 1),
                        )
                    sl = slice(j * CH, (j + 1) * CH)
                    nc.vector.tensor_copy(out=kT_sb[:, sl], in_=ps)
                    # v: compute vT (feature-major, fast N) then PE-transpose
                    # into natural [tokens, feats] bf16 tiles
                    psvt = pa_ps.tile([FPC, CH], f32, tag="ps", name=f"psvt{j}")
                    for kc in range(D // 128):
                        nc.tensor.matmul(
                            psvt,
                            wv_sb[:, kc, :],
                            kv_in[:, kc, :],
                            start=(kc == 0),
                            stop=(kc == D // 128 - 1),
                        )
                    vt_t = pa_in.tile([FPC, CH], pdt, tag="vt_t", name=f"vt_t{j}")
                    nc.vector.tensor_copy(out=vt_t, in_=psvt)
                    for s in range(CH // 128):
                        psv = pa_psv.tile([128, FPC], pdt, tag="psv", name=f"psv{j}_{s}")
                        nc.tensor.transpose(
                            psv,
                            vt_t[:, s * 128 : (s + 1) * 128],
                            ident_bf,
                        )
                        cidx = j * (CH // 128) + s
                        for hh in range(HPC):
                            nc.scalar.copy(
                                out=v_sb[:, cidx, hh, 0:DV],
                                in_=psv[:, hh * DV : (hh + 1) * DV],
                            )
                    r_in = pa_in.tile(
                        [128, D // 128, CH], pdt, tag="xin2", name=f"r_in{j}"
                    )
                    nc.scalar.dma_start(
                        out=r_in,
                        in_=xrT[:].rearrange("(kc p) t -> p kc t", p=128)[
                            :, :, j * CH : (j + 1) * CH
                        ],
                    )
                    ps2 = pa_ps.tile([FPC, CH], f32, tag="ps", name=f"ps_r{j}")
                    for kc in range(D // 128):
                        nc.tensor.matmul(
                            ps2,
                            wr_sb[:, kc, :],
                            r_in[:, kc, :],
                            start=(kc == 0),
                            stop=(kc == D // 128 - 1),
                        )
                    nc.vector.tensor_copy(out=rT_sb[:, sl], in_=ps2)

                for j in range(2):
                    emit_q_chunk(j)
                for j in range(4):
                    emit_kvr_chunk(j)
                for j in range(2, 4):
                    emit_q_chunk(j)
                for j in range(4, 8):
                    emit_kvr_chunk(j)
                pa_psv.release()
                pa_ps.release()
                pa_in.release()

                # attention pools
                pb_rows = tc.alloc_tile_pool(name="pb_rows", bufs=4)
                pb_bdt = tc.alloc_tile_pool(name="pb_bdt", bufs=4)
                pb_eac = tc.alloc_tile_pool(name="pb_eac", bufs=4)
                pb_prod = tc.alloc_tile_pool(name="pb_prod", bufs=4)
                pb_bc = tc.alloc_tile_pool(name="pb_bc", bufs=2)
                pb_small = tc.alloc_tile_pool(name="pb_small", bufs=2)
                pb_ps = tc.alloc_tile_pool(name="pb_ps", bufs=3, space="PSUM")
                pb_ps2 = tc.alloc_tile_pool(name="pb_ps2", bufs=2, space="PSUM")
                pb_ctx = tc.alloc_tile_pool(name="pb_ctx", bufs=2, space="PSUM")
                pb_psb = tc.alloc_tile_pool(name="pb_psb", bufs=1, space="PSUM")

                def bd_raw_stages(pi, b, hh, t):
                    # exp(bd/8) rows for q chunk t, q-major, unshifted.
                    # Returned as fine-grained thunks (one matmul+exp each,
                    # plus the DMA write) so they interleave with the score
                    # pipeline without bunching up on the scalar engine.
                    qf = slice(64 * hh, 64 * hh + 64)
                    n0 = 1 if t < 4 else 0
                    state = {}

                    def mk_mm(n):
                        def thunk():
                            if "row" not in state:
                                state["row"] = pb_rows.tile(
                                    [128, TKV],
                                    bf16,
                                    tag="bd_row",
                                    name=f"bd_row{pi}_{t}",
                                )
                            ps_bd = pb_ps.tile([128, 512], f32, tag="ps_sc")
                            nc.tensor.matmul(
                                ps_bd,
                                qpb_sb[
                                    qf, b * TQ + t * 128 : b * TQ + (t + 1) * 128
                                ],
                                rT_sb[
                                    qf, b * TKV + 512 * n : b * TKV + 512 * (n + 1)
                                ],
                                start=True,
                                stop=True,
                            )
                            nc.scalar.activation(
                                out=state["row"][:, 512 * n : 512 * (n + 1)],
                                in_=ps_bd,
                                func=Exp,
                                scale=0.125,
                            )

                        return thunk

                    def wr_thunk():
                        nc.gpsimd.dma_start(
                            out=AP(
                                tensor=bd_dram[pi][t // 4][:].tensor,
                                offset=(t % 4) * 128 * TKV + 512 * n0,
                                ap=[[TKV, 128], [1, TKV - 512 * n0]],
                            ),
                            in_=state["row"][:, 512 * n0 : TKV],
                        )

                    return [mk_mm(n) for n in range(n0, NK)] + [wr_thunk]

                def attn_half(pi, b, hh, h, fillers, pending_finish):
                    qf = slice(64 * hh, 64 * hh + 64)
                    kcmax = 12 + 4 * h
                    ps_ctx = pb_ctx.tile(
                        [DV + 1, 512], f32, tag="ps_ctx", name=f"psctx{pi}_{h}"
                    )
                    bd_tiles = {}
                    prod_tiles = {}

                    def issue_read(kcp):
                        # shifted+transposed exp(bd) tiles for kc pair
                        # (2kcp, 2kcp+1): [kv 128, 2, q 512] via one XBAR read;
                        # only rows covering the unmasked q range are read.
                        qlo = max(0, 128 * (2 * kcp - 8) - 512 * h)
                        ebdT = pb_bdt.tile([128, 2, 512], bf16, tag="ebdT")
                        nc.sync.dma_start(
                            out=ebdT[:, :, qlo:512],
                            in_=AP(
                                tensor=bd_dram[pi][h][:].tensor,
                                offset=(TQ - 1 - 512 * h)
                                + 256 * kcp
                                + qlo * (TKV - 1),
                                ap=[[TKV - 1, 512 - qlo], [1, 256]],
                            ),
                            transpose=True,
                        )
                        bd_tiles[kcp] = ebdT

                    def score_stage(kc):
                        eng = nc.vector
                        bdt = bd_tiles[kc // 2][:, kc % 2, :]
                        if kc >= 8 + 4 * h:
                            # keep where q >= k - R_OFF, i.e.
                            # j + (512h + R_OFF - 128 kc) - p >= 0
                            nc.gpsimd.affine_select(
                                out=bdt,
                                in_=bdt,
                                pattern=[[1, 512]],
                                compare_op=ALU.is_ge,
                                fill=0.0,
                                base=512 * h + R_OFF - 128 * kc,
                                channel_multiplier=-1,
                            )
                        ps_sc = pb_ps2.tile([128, 512], f32, tag="ps_sc2")
                        nc.tensor.matmul(
                            ps_sc,
                            kT_sb[qf, b * TKV + 128 * kc : b * TKV + 128 * (kc + 1)],
                            qcb_sb[qf, b * TQ + 512 * h : b * TQ + 512 * (h + 1)],
                            start=True,
                            stop=True,
                        )
                        eacT = pb_eac.tile([128, 512], bf16, tag="eacT")
                        nc.scalar.activation(
                            out=eacT, in_=ps_sc, func=Exp, scale=0.125
                        )
                        expT = pb_prod.tile([128, 512], bf16, tag="expT")
                        eng.tensor_mul(out=expT, in0=eacT, in1=bdt)
                        if kc % 2 == 1:
                            bd_tiles.pop(kc // 2)
                        prod_tiles[kc] = expT

                    def ctx_stage(kc):
                        nc.tensor.matmul(
                            ps_ctx,
                            v_sb[:, b * (TKV // 128) + kc, hh, :],
                            prod_tiles.pop(kc),
                            start=(kc == 0),
                            stop=(kc == kcmax - 1),
                        )

                    for k2 in range(2):
                        issue_read(k2)
                    for kc in range(kcmax):
                        if kc % 2 == 0 and kc // 2 + 2 < kcmax // 2:
                            issue_read(kc // 2 + 2)
                        score_stage(kc)
                        if kc >= 2:
                            ctx_stage(kc - 2)
                        if kc == 1 and pending_finish is not None:
                            pending_finish()
                            pending_finish = None
                        # spread filler stages so the scalar engine never
                        # queues bd-exp work ahead of the critical eacT exp
                        slots_left = kcmax - kc
                        take = (len(fillers) + slots_left - 1) // slots_left
                        for _ in range(min(take, len(fillers))):
                            fillers.pop(0)()
                    ctx_stage(kcmax - 2)
                    ctx_stage(kcmax - 1)
                    if pending_finish is not None:
                        pending_finish()

                    def finish():
                        # normalize: ctxT[f, q] *= 1/den[q], den in psum row DV
                        recip = pb_small.tile([1, 512], bf16, tag="recip")
                        with nc.allow_low_precision(
                            reason="bf16 1/denominator matches baseline attn bf16"
                        ):
                            nc.vector.reciprocal(recip, ps_ctx[DV : DV + 1, :])
                        ps_b = pb_psb.tile([DV, 512], f32, tag="ps_b")
                        nc.tensor.matmul(
                            ps_b, ones_bf, recip, start=True, stop=True
                        )
                        bcast = pb_bc.tile([DV, 512], f32, tag="bcast")
                        nc.scalar.copy(out=bcast, in_=ps_b)
                        nc.vector.tensor_mul(
                            out=ctx_sb[
                                qf, b * TQ + 512 * h : b * TQ + 512 * (h + 1)
                            ],
                            in0=ps_ctx[0:DV, :],
                            in1=bcast,
                        )

                    return finish

                pairs = [(0, 0, 0), (1, 0, 1), (2, 1, 0), (3, 1, 1)]
                pending = None
                for idx, (pi, b, hh) in enumerate(pairs):
                    if idx == 0:
                        for t in range(4):
                            for th in bd_raw_stages(pi, b, hh, t):
                                th()
                    fill0 = []
                    for t in range(4):
                        fill0 += bd_raw_stages(pi, b, hh, 4 + t)
                    pending = attn_half(pi, b, hh, 0, fill0, pending)
                    fill1 = []
                    if idx + 1 < 4:
                        pj, bj, hj = pairs[idx + 1]
                        for t in range(4):
                            fill1 += bd_raw_stages(pj, bj, hj, t)
                    pending = attn_half(pi, b, hh, 1, fill1, pending)
                if pending is not None:
                    pending()

                pb_psb.release()
                pb_ctx.release()
                pb_ps2.release()
                pb_ps.release()
                pb_small.release()
                pb_bc.release()
                pb_prod.release()
                pb_eac.release()
                pb_bdt.release()
                pb_rows.release()

            # ================= Phase C: exchange + output projection + LN =========
            nc.sync.dma_start(
                out=a2a_in[:].rearrange("(j p) t -> p j t", p=FPC),
                in_=ctx_sb[:].rearrange("p (j t) -> p j t", t=RPC),
            )
            if n_cores > 1:
                nc.gpsimd.collective_compute(
                    "AllToAll",
                    ALU.bypass,
                    replica_groups=[list(range(n_cores))],
                    ins=[a2a_in[:]],
                    outs=[a2a_out[:]],
                )
            else:
                # single-core profiling variant: plain copy instead
                nc.sync.dma_start(out=a2a_out[:], in_=a2a_in[:])
            with tc.tile_pool(name="pc", bufs=3) as pc, tc.tile_pool(
                name="pc_ps", bufs=4, space="PSUM"
            ) as pc_ps, tc.tile_pool(name="pc_small", bufs=8) as pc_small:
                for mc in range(RPC // 128):
                    ps_o = [pc_ps.tile([128, 512], f32, tag="ps_o", name=f"ps_o{nn_}") for nn_ in range(2)]
                    for kc in range(D // 128):
                        lhs = pc.tile([128, 128], pdt, tag="octx")
                        nc.sync.dma_start(
                            out=lhs,
                            in_=a2a_out[
                                kc * 128 : (kc + 1) * 128, mc * 128 : (mc + 1) * 128
                            ],
                        )
                        for nn in range(2):
                            nc.tensor.matmul(
                                ps_o[nn],
                                lhs,
                                wo_sb[:, kc, nn * 512 : (nn + 1) * 512],
                                start=(kc == 0),
                                stop=(kc == D // 128 - 1),
                            )
                    o_sb = pc.tile([128, D], f32, tag="o_sb")
                    for nn in range(2):
                        nc.vector.tensor_add(
                            out=o_sb[:, nn * 512 : (nn + 1) * 512],
                            in0=ps_o[nn],
                            in1=qres_sb[:, mc, nn * 512 : (nn + 1) * 512],
                        )
                    # LayerNorm over the free (feature) dim
                    stats = pc_small.tile([128, 2, 6], f32, tag="stats")
                    for sg in range(2):
                        nc.vector.bn_stats(
                            out=stats[:, sg, :], in_=o_sb[:, sg * 512 : (sg + 1) * 512]
                        )
                    mv = pc_small.tile([128, 2], f32, tag="mv")
                    nc.vector.bn_aggr(out=mv, in_=stats)
                    mean, var = mv[:, 0:1], mv[:, 1:2]
                    xve = pc_small.tile([128, 1], f32, tag="xve")
                    nc.vector.tensor_scalar_add(out=xve, in0=var, scalar1=eps_sb)
                    std = pc_small.tile([128, 1], f32, tag="std")
                    nc.scalar.activation(out=std, in_=var, func=Sqrt, bias=eps_sb)
                    rstd = pc_small.tile([128, 1], f32, tag="rstd")
                    nc.vector.reciprocal(rstd, std)
                    # one Newton step for rsqrt accuracy:
                    # r <- r * (1.5 - 0.5 * x * r^2)
                    tnw = pc_small.tile([128, 1], f32, tag="tnw")
                    nc.vector.tensor_mul(out=tnw, in0=rstd, in1=rstd)
                    nc.vector.tensor_mul(out=tnw, in0=tnw, in1=xve)
                    nc.vector.tensor_scalar(
                        out=tnw, in0=tnw, scalar1=-0.5, scalar2=1.5,
                        op0=ALU.mult, op1=ALU.add,
                    )
                    nc.vector.tensor_scalar_mul(out=rstd, in0=rstd, scalar1=tnw)
                    nc.vector.tensor_scalar(
                        out=o_sb, in0=o_sb, scalar1=mean, scalar2=rstd,
                        op0=ALU.subtract, op1=ALU.mult,
                    )
                    nc.vector.tensor_mul(out=o_sb, in0=o_sb, in1=gamma_sb)
                    nc.vector.tensor_add(out=o_sb, in0=o_sb, in1=beta_sb)
                    nc.sync.dma_start(
                        out=out[mc * 128 : (mc + 1) * 128, :], in_=o_sb
                    )
    return nc


def _make_in_maps(inputs, mm_dtype="bfloat16"):
    query = np.asarray(inputs["query"], np.float32)
    key_value = np.asarray(inputs["key_value"], np.float32)
    relative = np.asarray(inputs["relative"], np.float32)
    content_bias = np.asarray(inputs["content_bias"], np.float32)
    position_bias = np.asarray(inputs["position_bias"], np.float32)
    Wq, Wk = np.asarray(inputs["Wq"], np.float32), np.asarray(inputs["Wk"], np.float32)
    Wv, Wr = np.asarray(inputs["Wv"], np.float32), np.asarray(inputs["Wr"], np.float32)
    Wo = np.ascontiguousarray(np.asarray(inputs["Wo"], np.float32))
    ln_gamma = np.asarray(inputs["ln_gamma"], np.float32)
    ln_beta = np.asarray(inputs["ln_beta"], np.float32)

    qflat = query.reshape(B * TQ, D)
    if mm_dtype == "bfloat16":
        import ml_dtypes

        mdt = ml_dtypes.bfloat16
    else:
        mdt = np.float32
    xqT = np.ascontiguousarray(qflat.T).astype(mdt)
    xkvT = np.ascontiguousarray(key_value.reshape(B * TKV, D).T).astype(mdt)
    xrT = np.ascontiguousarray(relative.reshape(B * TKV, D).T).astype(mdt)
    Wq, Wk = Wq.astype(mdt), Wk.astype(mdt)
    Wv, Wr = Wv.astype(mdt), Wr.astype(mdt)
    Wo = Wo.astype(mdt)
    cb = content_bias.reshape(NH, DV)
    pb = position_bias.reshape(NH, DV)

    in_maps = []
    for c in range(N_CORES):
        fs = slice(FPC * c, FPC * (c + 1))
        in_maps.append(
            {
                "xqT": xqT,
                "xkvT": xkvT,
                "xrT": xrT,
                "wq": np.ascontiguousarray(Wq[:, fs]),
                "wk": np.ascontiguousarray(Wk[:, fs]),
                "wv": np.ascontiguousarray(Wv[:, fs]),
                "wr": np.ascontiguousarray(Wr[:, fs]),
                "wo": Wo,
                "cbv": np.ascontiguousarray(
                    cb[HPC * c : HPC * (c + 1)].reshape(FPC, 1)
                ),
                "pbv": np.ascontiguousarray(
                    pb[HPC * c : HPC * (c + 1)].reshape(FPC, 1)
                ),
                "qres": np.ascontiguousarray(qflat[RPC * c : RPC * (c + 1)]),
                "gamma": ln_gamma,
                "beta": ln_beta,
            }
        )
    return in_maps


def run_on_hw(inputs, trace=False, score_dtype="bfloat16", proj_dtype="bfloat16"):
    from concourse.bass_utils import run_bass_kernel_spmd

    key = (score_dtype, proj_dtype)
    nc = _CACHE.get(key)
    if nc is None:
        nc = build_program(score_dtype=score_dtype, proj_dtype=proj_dtype)
        _CACHE[key] = nc
    in_maps = _make_in_maps(inputs, mm_dtype=proj_dtype)
    res = run_bass_kernel_spmd(nc, in_maps, list(range(N_CORES)), trace=trace)
    outs = np.concatenate(
        [np.asarray(res.results[c]["out"]) for c in range(N_CORES)], axis=0
    )
    return outs.reshape(B, TQ, D), res


def kernel(**inputs) -> np.ndarray:
    out, _ = run_on_hw(inputs)
    return out


# revision 31
# speedup vs baseline: 1.4201x; 1.0463x over previous
"""Trainium2 Bass kernel for Transformer-XL style relative-position attention.

Reference computation (B=2, Tq=1024, Tkv=2048, D=1024, H=16, Dv=64):
    q/k/v/r projections, ac = (q+cb)@k^T, bd = rel_shift((q+pb)@r^T),
    softmax((ac+bd)/8) with causal-with-memory mask, ctx = attn@v,
    out = LN(ctx@Wo + query).

Sharding (Megatron-style tensor parallel over heads, 8 cores):
  - each core owns 2 heads: Wq/Wk/Wv/Wr column shards [1024,128], biases
    shard with heads.
  - activations (transposed on host to feature-major) are broadcast.
  - after per-head attention, ctx^T shards are exchanged with a single
    AllToAll so that each core ends up with the full ctx^T for 1/8 of the
    token rows; each core then does that row-slice of ctx@Wo + residual +
    LayerNorm with the full Wo.

Device-side structure (transposed-scores design):
  - scores are computed TRANSPOSED (kv on partitions, q on the free dim):
    acT tiles come straight from a matmul with kT as the stationary
    operand; this makes the softmax output directly consumable by the
    attn@v contraction with NO attention-matrix transpose or DRAM
    round-trip.
  - rel_shift: bd is computed q-major (dense PE work), EXPONENTIATED
    (exp commutes with the shift), written bf16 to a flat DRAM scratch,
    and read back through a strided AP with row stride Tkv-1 PLUS
    transpose=True (hardware XBAR transpose), which lands the *shifted,
    transposed* exp(bd/8) tiles in SBUF in one step.
  - softmax numerator: exp((ac+bd)/8) = exp(ac/8) * exp(bd/8): the
    scalar engine exponentiates acT from PSUM, and the product runs on
    gpsimd/vector (SBUF-only operands), so no engine ever needs an
    (ac+bd) add against PSUM.
  - causal mask applied to the exp(bd) tiles with affine_select fill=0.
  - softmax denominators come for free from a ones-column appended to v:
    the attn@v matmul accumulates sum(exp) in psum row 64.
  - 1/denominator (per q) is broadcast across the 64 feature partitions
    with a rank-1 matmul (ones ⊗ recip) and applied to the small ctx^T
    tile instead of the big attention matrix.
"""

import numpy as np

# problem shapes (hardcoded per contract)
B, TQ, TKV, D, NH, DV = 2, 1024, 2048, 1024, 16, 64
N_CORES = 8
HPC = NH // N_CORES          # heads per core = 2
FPC = HPC * DV               # head-feature columns per core = 128
RPC = (B * TQ) // N_CORES    # output token rows per core = 256
R_OFF = TKV - TQ             # causal memory offset = 1024
LN_EPS = 1e-5
NT = TQ // 128               # query row chunks = 8
NK = TKV // 512              # key col chunks of 512 = 4

_CACHE = {}


def _patched_tc_class():
    """TileContext whose kernel-tail drain splits sem waits one per drain.

    The walrus build in this container rejects CTRL-type instructions
    (InstDrain) carrying more than one sync-wait command.
    """
    import concourse.mybir as mybir
    import concourse.tile as tile
    from concourse.vector_clock import ScopedClock

    class TC(tile.TileContext):
        def _commit_instruction(self, inst, lazy_reg_writes=True):
            # This walrus build rejects instructions carrying more than one
            # sync-wait command; hoist extras onto preceding NoOp carriers.
            si = getattr(inst, "sync_info", None)
            if (
                si is not None
                and si.on_wait
                and len(si.on_wait) > 1
                and inst.engine != mybir.EngineType.Unassigned
            ):
                waits = list(si.on_wait)
                inst.sync_info = mybir.SyncInfo(
                    on_wait=[waits[-1]], on_update=list(si.on_update or [])
                )
                for w in waits[:-1]:
                    ev = mybir.InstNoOp(
                        name=f"I-wsplit-{self.nc.next_id()}", ins=[], outs=[]
                    )
                    ev.engine = inst.engine
                    ev.sync_info = mybir.SyncInfo(on_wait=[w], on_update=[])
                    self._add_instruction(ev)
            return super()._commit_instruction(inst, lazy_reg_writes)

        def _drain_and_barrier(self, tick_clock, wait_clock):
            nc = self.nc
            drain_inst = nc.sync.drain()
            wait_clock.add_sem_waits(
                drain_inst.ins, ScopedClock({None: tick_clock.global_clock})
            )
            inner = drain_inst.ins
            si = inner.sync_info
            waits = list(si.on_wait) if si and si.on_wait else []
            if len(waits) > 1:
                inner.sync_info = mybir.SyncInfo(
                    on_wait=waits[:1], on_update=list(si.on_update or [])
                )
                for w in waits[1:]:
                    d2 = nc.sync.drain()
                    d2.ins.sync_info = mybir.SyncInfo(on_wait=[w], on_update=[])
            nc.all_engine_barrier()
            assert self.sems is not None
            popped = nc._tile_sem_poison_stack.pop()
            assert popped is self._sem_poison
            nc.clear_and_free_semaphores(list(self.sems.allocated().values()))
            nc.all_engine_barrier()

    return TC


def build_program(score_dtype="bfloat16", proj_dtype="bfloat16", n_cores=N_CORES):
    """Build the SPMD Bass program (identical on all 8 cores).

    n_cores=1 builds a single-core variant (collective replaced by a
    self-copy) for profiling; its output is only valid for core 0's
    feature shard.
    """
    import concourse.bass as bass
    import concourse.mybir as mybir
    from concourse.bass import AP

    f32 = mybir.dt.float32
    bf16 = mybir.dt.bfloat16
    pdt = bf16
    sdt = bf16
    TC = _patched_tc_class()

    nc = bass.Bass()

    # ---- I/O ----
    xqT = nc.dram_tensor("xqT", [D, B * TQ], pdt, kind="ExternalInput")
    xkvT = nc.dram_tensor("xkvT", [D, B * TKV], pdt, kind="ExternalInput")
    xrT = nc.dram_tensor("xrT", [D, B * TKV], pdt, kind="ExternalInput")
    wq = nc.dram_tensor("wq", [D, FPC], pdt, kind="ExternalInput")
    wk = nc.dram_tensor("wk", [D, FPC], pdt, kind="ExternalInput")
    wv = nc.dram_tensor("wv", [D, FPC], pdt, kind="ExternalInput")
    wr = nc.dram_tensor("wr", [D, FPC], pdt, kind="ExternalInput")
    wo = nc.dram_tensor("wo", [D, D], pdt, kind="ExternalInput")
    cbv = nc.dram_tensor("cbv", [FPC, 1], f32, kind="ExternalInput")
    pbv = nc.dram_tensor("pbv", [FPC, 1], f32, kind="ExternalInput")
    qres = nc.dram_tensor("qres", [RPC, D], f32, kind="ExternalInput")
    gamma = nc.dram_tensor("gamma", [D], f32, kind="ExternalInput")
    beta = nc.dram_tensor("beta", [D], f32, kind="ExternalInput")
    out = nc.dram_tensor("out", [RPC, D], f32, kind="ExternalOutput")

    # ---- internal DRAM scratch ----
    # raw bd per (pair, q-half): flat [512 rows x TKV]; the shifted,
    # transposed read only ever depends on its own half's rows.
    bd_dram = [
        [nc.dram_tensor(f"bd_dram{p}_{h}", [512 * TKV], bf16) for h in range(2)]
        for p in range(4)
    ]
    # one AllToAll per batch: core c owns q rows [128c, 128c+128) of EACH
    # batch, so batch-0 exchange + output projection overlap batch-1 attention
    a2a_in = [nc.dram_tensor(f"a2a_in{b}", [N_CORES * FPC, TQ // 8], pdt) for b in range(B)]
    a2a_out = [nc.dram_tensor(f"a2a_out{b}", [N_CORES * FPC, TQ // 8], pdt) for b in range(B)]

    Exp = mybir.ActivationFunctionType.Exp
    Identity = mybir.ActivationFunctionType.Identity
    Sqrt = mybir.ActivationFunctionType.Sqrt
    ALU = mybir.AluOpType

    with TC(nc) as tc:
        import contextlib

        with contextlib.ExitStack() as ctx:
            singles = ctx.enter_context(tc.tile_pool(name="singles", bufs=1))

            # ---- static SBUF tensors ----
            wq_sb = singles.tile([128, D // 128, FPC], pdt, tag="wq_sb")
            wk_sb = singles.tile([128, D // 128, FPC], pdt, tag="wk_sb")
            wv_sb = singles.tile([128, D // 128, FPC], pdt, tag="wv_sb")
            wr_sb = singles.tile([128, D // 128, FPC], pdt, tag="wr_sb")
            for w_sb, w_dr in ((wq_sb, wq), (wk_sb, wk), (wv_sb, wv), (wr_sb, wr)):
                nc.gpsimd.dma_start(
                    out=w_sb, in_=w_dr[:].rearrange("(kc p) f -> p kc f", p=128)
                )
            wo_sb = singles.tile([128, D // 128, D], pdt, tag="wo_sb")
            nc.gpsimd.dma_start(
                out=wo_sb, in_=wo[:].rearrange("(kc p) d -> p kc d", p=128)
            )
            cb_sb = singles.tile([FPC, 1], f32, tag="cb_sb")
            pb_sb = singles.tile([FPC, 1], f32, tag="pb_sb")
            nc.sync.dma_start(out=cb_sb, in_=cbv[:])
            nc.sync.dma_start(out=pb_sb, in_=pbv[:])
            eps_sb = singles.tile([128, 1], f32, tag="eps_sb")
            nc.vector.memset(eps_sb, LN_EPS)
            gamma_sb = singles.tile([128, D], f32, tag="gamma_sb")
            beta_sb = singles.tile([128, D], f32, tag="beta_sb")
            nc.gpsimd.dma_start(
                out=gamma_sb,
                in_=AP(tensor=gamma[:].tensor, offset=0, ap=[[0, 128], [1, D]]),
            )
            nc.gpsimd.dma_start(
                out=beta_sb,
                in_=AP(tensor=beta[:].tensor, offset=0, ap=[[0, 128], [1, D]]),
            )
            qres_sb = singles.tile([128, RPC // 128, D], f32, tag="qres_sb")
            nc.gpsimd.dma_start(
                out=qres_sb, in_=qres[:].rearrange("(mc p) d -> p mc d", p=128)
            )

            # projection outputs (feature-major, both heads stacked on partitions)
            qcb_sb = singles.tile([FPC, B * TQ], sdt, tag="qcb_sb")
            qpb_sb = singles.tile([FPC, B * TQ], sdt, tag="qpb_sb")
            kT_sb = singles.tile([FPC, B * TKV], sdt, tag="kT_sb")
            rT_sb = singles.tile([FPC, B * TKV], sdt, tag="rT_sb")
            # v in natural layout [kv-token partitions, chunk, head, 64+ones]
            v_sb = singles.tile([128, (B * TKV) // 128, HPC, DV + 1], bf16, tag="v_sb")
            nc.vector.memset(v_sb[:, :, :, DV], 1.0)
            ctx_sb = singles.tile([FPC, B * TQ], pdt, tag="ctx_sb")
            ones_bf = singles.tile([1, DV], bf16, tag="ones_bf")
            nc.vector.memset(ones_bf, 1.0)
            # row-selector for the 1/den broadcast matmul: column block r
            # of sel_bf picks the denominator row at partition 32r
            sel_bf = singles.tile([128, 4 * DV], bf16, tag="sel_bf")
            nc.vector.memset(sel_bf, 0.0)
            for r_ in range(4):
                nc.scalar.copy(
                    out=sel_bf[32 * r_ : 32 * r_ + 1, r_ * DV : (r_ + 1) * DV],
                    in_=ones_bf,
                )
            # softmax denominators at partitions 0/32/64/96, one row per
            # (pair-in-batch, half); unused partitions stay 1.0 so the
            # batch-wide reciprocal never produces inf/nan
            den_sb = singles.tile([128, B, 512], f32, tag="den_sb")
            nc.vector.memset(den_sb, 1.0)
            recip_sb = singles.tile([128, B, 512], bf16, tag="recip_sb")

            # identity (bf16) for PE-transposes
            ident_bf = singles.tile([128, 128], bf16, tag="ident_bf")
            from concourse.masks import make_identity

            make_identity(nc, ident_bf)

            # ========== Phases A+B interleaved: projections + attention ==========
            CH = 512  # token columns per projection step
            with contextlib.ExitStack() as phase_ab:
                pa_in = tc.alloc_tile_pool(name="pa_in", bufs=2)
                pa_ps = tc.alloc_tile_pool(name="pa_ps", bufs=4, space="PSUM")
                pa_psv = tc.alloc_tile_pool(name="pa_psv", bufs=4, space="PSUM")

                def emit_q_chunk(j):
                    q_in = pa_in.tile(
                        [128, D // 128, CH], pdt, tag="xin", name=f"q_in{j}"
                    )
                    nc.sync.dma_start(
                        out=q_in,
                        in_=xqT[:].rearrange("(kc p) t -> p kc t", p=128)[
                            :, :, j * CH : (j + 1) * CH
                        ],
                    )
                    ps = pa_ps.tile([FPC, CH], f32, tag="ps", name=f"ps_q{j}")
                    for kc in range(D // 128):
                        nc.tensor.matmul(
                            ps,
                            wq_sb[:, kc, :],
                            q_in[:, kc, :],
                            start=(kc == 0),
                            stop=(kc == D // 128 - 1),
                        )
                    sl = slice(j * CH, (j + 1) * CH)
                    nc.vector.tensor_scalar_add(
                        out=qcb_sb[:, sl], in0=ps, scalar1=cb_sb
                    )
                    nc.vector.tensor_scalar_add(
                        out=qpb_sb[:, sl], in0=ps, scalar1=pb_sb
                    )

                def emit_kvr_chunk(j):
                    kv_in = pa_in.tile(
                        [128, D // 128, CH], pdt, tag="xin", name=f"kv_in{j}"
                    )
                    nc.sync.dma_start(
                        out=kv_in,
                        in_=xkvT[:].rearrange("(kc p) t -> p kc t", p=128)[
                            :, :, j * CH : (j + 1) * CH
                        ],
                    )
                    ps = pa_ps.tile([FPC, CH], f32, tag="ps", name=f"ps_k{j}")
                    for kc in range(D // 128):
                        nc.tensor.matmul(
                            ps,
                            wk_sb[:, kc, :],
                            kv_in[:, kc, :],
                            start=(kc == 0),
                            stop=(kc == D // 128 - 1),
                        )
                    sl = slice(j * CH, (j + 1) * CH)
                    nc.vector.tensor_copy(out=kT_sb[:, sl], in_=ps)
                    # v: compute vT (feature-major, fast N) then PE-transpose
                    # into natural [tokens, feats] bf16 tiles
                    psvt = pa_ps.tile([FPC, CH], f32, tag="ps", name=f"psvt{j}")
                    for kc in range(D // 128):
                        nc.tensor.matmul(
                            psvt,
                            wv_sb[:, kc, :],
                            kv_in[:, kc, :],
                            start=(kc == 0),
                            stop=(kc == D // 128 - 1),
                        )
                    vt_t = pa_in.tile([FPC, CH], pdt, tag="vt_t", name=f"vt_t{j}")
                    nc.vector.tensor_copy(out=vt_t, in_=psvt)
                    for s in range(CH // 128):
                        psv = pa_psv.tile([128, FPC], pdt, tag="psv", name=f"psv{j}_{s}")
                        nc.tensor.transpose(
                            psv,
                            vt_t[:, s * 128 : (s + 1) * 128],
                            ident_bf,
                        )
                        cidx = j * (CH // 128) + s
                        for hh in range(HPC):
                            nc.scalar.copy(
                                out=v_sb[:, cidx, hh, 0:DV],
                                in_=psv[:, hh * DV : (hh + 1) * DV],
                            )
                    r_in = pa_in.tile(
                        [128, D // 128, CH], pdt, tag="xin2", name=f"r_in{j}"
                    )
                    nc.scalar.dma_start(
                        out=r_in,
                        in_=xrT[:].rearrange("(kc p) t -> p kc t", p=128)[
                            :, :, j * CH : (j + 1) * CH
                        ],
                    )
                    ps2 = pa_ps.tile([FPC, CH], f32, tag="ps", name=f"ps_r{j}")
                    for kc in range(D // 128):
                        nc.tensor.matmul(
                            ps2,
                            wr_sb[:, kc, :],
                            r_in[:, kc, :],
                            start=(kc == 0),
                            stop=(kc == D // 128 - 1),
                        )
                    nc.vector.tensor_copy(out=rT_sb[:, sl], in_=ps2)

                for j in range(2):
                    emit_q_chunk(j)
                for j in range(4):
                    emit_kvr_chunk(j)
                for j in range(2, 4):
                    emit_q_chunk(j)
                for j in range(4, 8):
                    emit_kvr_chunk(j)
                pa_psv.release()
                pa_ps.release()
                pa_in.release()

                # attention pools
                pb_rows = tc.alloc_tile_pool(name="pb_rows", bufs=4)
                pb_bdt = tc.alloc_tile_pool(name="pb_bdt", bufs=4)
                pb_eac = tc.alloc_tile_pool(name="pb_eac", bufs=4)
                pb_prod = tc.alloc_tile_pool(name="pb_prod", bufs=4)
                pb_bc = tc.alloc_tile_pool(name="pb_bc", bufs=2)
                pb_small = tc.alloc_tile_pool(name="pb_small", bufs=2)
                pb_ps = tc.alloc_tile_pool(name="pb_ps", bufs=2, space="PSUM")
                pb_ps2 = tc.alloc_tile_pool(name="pb_ps2", bufs=2, space="PSUM")
                pb_ctx = tc.alloc_tile_pool(name="pb_ctx", bufs=2, space="PSUM")

                def bd_raw_stages(pi, b, hh, t):
                    # exp(bd/8) rows for q chunk t, q-major, unshifted.
                    # Returned as fine-grained thunks (one matmul+exp each,
                    # plus the DMA write) so they interleave with the score
                    # pipeline without bunching up on the scalar engine.
                    qf = slice(64 * hh, 64 * hh + 64)
                    n0 = 1 if t < 4 else 0
                    state = {}

                    def mk_mm(n):
                        def thunk():
                            if "row" not in state:
                                state["row"] = pb_rows.tile(
                                    [128, TKV],
                                    bf16,
                                    tag="bd_row",
                                    name=f"bd_row{pi}_{t}",
                                )
                            ps_bd = pb_ps.tile([128, 512], f32, tag="ps_sc")
                            nc.tensor.matmul(
                                ps_bd,
                                qpb_sb[
                                    qf, b * TQ + t * 128 : b * TQ + (t + 1) * 128
                                ],
                                rT_sb[
                                    qf, b * TKV + 512 * n : b * TKV + 512 * (n + 1)
                                ],
                                start=True,
                                stop=True,
                            )
                            nc.scalar.activation(
                                out=state["row"][:, 512 * n : 512 * (n + 1)],
                                in_=ps_bd,
                                func=Exp,
                                scale=0.125,
                            )

                        return thunk

                    def wr_thunk():
                        nc.gpsimd.dma_start(
                            out=AP(
                                tensor=bd_dram[pi][t // 4][:].tensor,
                                offset=(t % 4) * 128 * TKV + 512 * n0,
                                ap=[[TKV, 128], [1, TKV - 512 * n0]],
                            ),
                            in_=state["row"][:, 512 * n0 : TKV],
                        )

                    return [mk_mm(n) for n in range(n0, NK)] + [wr_thunk]

                def attn_half(pi, b, hh, h, fillers, pending_finish):
                    qf = slice(64 * hh, 64 * hh + 64)
                    kcmax = 12 + 4 * h
                    ps_ctx = pb_ctx.tile(
                        [DV + 1, 512], f32, tag="ps_ctx", name=f"psctx{pi}_{h}"
                    )
                    bd_tiles = {}
                    prod_tiles = {}

                    def issue_read(kcp):
                        # shifted+transposed exp(bd) tiles for kc pair
                        # (2kcp, 2kcp+1): [kv 128, 2, q 512] via one XBAR read;
                        # only rows covering the unmasked q range are read.
                        qlo = max(0, 128 * (2 * kcp - 8) - 512 * h)
                        ebdT = pb_bdt.tile([128, 2, 512], bf16, tag="ebdT")
                        nc.sync.dma_start(
                            out=ebdT[:, :, qlo:512],
                            in_=AP(
                                tensor=bd_dram[pi][h][:].tensor,
                                offset=(TQ - 1 - 512 * h)
                                + 256 * kcp
                                + qlo * (TKV - 1),
                                ap=[[TKV - 1, 512 - qlo], [1, 256]],
                            ),
                            transpose=True,
                        )
                        bd_tiles[kcp] = ebdT

                    def score_stage(kc):
                        eng = nc.vector
                        bdt = bd_tiles[kc // 2][:, kc % 2, :]
                        if kc >= 8 + 4 * h:
                            # keep where q >= k - R_OFF, i.e.
                            # j + (512h + R_OFF - 128 kc) - p >= 0
                            nc.gpsimd.affine_select(
                                out=bdt,
                                in_=bdt,
                                pattern=[[1, 512]],
                                compare_op=ALU.is_ge,
                                fill=0.0,
                                base=512 * h + R_OFF - 128 * kc,
                                channel_multiplier=-1,
                            )
                        ps_sc = pb_ps2.tile([128, 512], f32, tag="ps_sc2")
                        nc.tensor.matmul(
                            ps_sc,
                            kT_sb[qf, b * TKV + 128 * kc : b * TKV + 128 * (kc + 1)],
                            qcb_sb[qf, b * TQ + 512 * h : b * TQ + 512 * (h + 1)],
                            start=True,
                            stop=True,
                        )
                        eacT = pb_eac.tile([128, 512], bf16, tag="eacT")
                        nc.scalar.activation(
                            out=eacT, in_=ps_sc, func=Exp, scale=0.125
                        )
                        expT = pb_prod.tile([128, 512], bf16, tag="expT")
                        eng.tensor_mul(out=expT, in0=eacT, in1=bdt)
                        if kc % 2 == 1:
                            bd_tiles.pop(kc // 2)
                        prod_tiles[kc] = expT

                    def ctx_stage(kc):
                        nc.tensor.matmul(
                            ps_ctx,
                            v_sb[:, b * (TKV // 128) + kc, hh, :],
                            prod_tiles.pop(kc),
                            start=(kc == 0),
                            stop=(kc == kcmax - 1),
                        )

                    for k2 in range(2):
                        issue_read(k2)
                    for kc in range(kcmax):
                        if kc % 2 == 0 and kc // 2 + 2 < kcmax // 2:
                            issue_read(kc // 2 + 2)
                        score_stage(kc)
                        if kc >= 2:
                            ctx_stage(kc - 2)
                        if kc == 1 and pending_finish is not None:
                            pending_finish()
                            pending_finish = None
                        # spread filler stages so the scalar engine never
                        # queues bd-exp work ahead of the critical eacT exp
                        slots_left = kcmax - kc
                        take = (len(fillers) + slots_left - 1) // slots_left
                        for _ in range(min(take, len(fillers))):
                            fillers.pop(0)()
                    ctx_stage(kcmax - 2)
                    ctx_stage(kcmax - 1)
                    if pending_finish is not None:
                        pending_finish()

                    def finish():
                        # stash denominator row + unnormalized ctx (bf16);
                        # the actual 1/den normalize happens per batch.
                        ridx = 2 * (pi % 2) + h
                        nc.scalar.copy(
                            out=den_sb[32 * ridx : 32 * ridx + 1, b, :],
                            in_=ps_ctx[DV : DV + 1, :],
                        )
                        nc.vector.tensor_copy(
                            out=ctx_sb[
                                qf, b * TQ + 512 * h : b * TQ + 512 * (h + 1)
                            ],
                            in_=ps_ctx[0:DV, :],
                        )

                    return finish

                def normalize_batch(b):
                    with nc.allow_low_precision(
                        reason="bf16 1/denominator matches baseline attn bf16"
                    ):
                        nc.vector.reciprocal(
                            recip_sb[:, b, :], den_sb[:, b, :]
                        )
                    for ridx in range(4):
                        hh = ridx // 2
                        h = ridx % 2
                        qf = slice(64 * hh, 64 * hh + 64)
                        cols = slice(b * TQ + 512 * h, b * TQ + 512 * (h + 1))
                        ps_b = pb_ctx.tile(
                            [DV, 512], f32, tag="ps_ctx", name=f"ps_b{b}_{ridx}"
                        )
                        nc.tensor.matmul(
                            ps_b,
                            sel_bf[:, ridx * DV : (ridx + 1) * DV],
                            recip_sb[:, b, :],
                            start=True,
                            stop=True,
                        )
                        bcast = pb_bc.tile(
                            [128, 512], bf16, tag="bcast", name=f"bc{b}_{ridx}"
                        )
                        nc.scalar.copy(out=bcast[qf, :], in_=ps_b)
                        nc.vector.tensor_mul(
                            out=ctx_sb[qf, cols],
                            in0=ctx_sb[qf, cols],
                            in1=bcast[qf, :],
                        )

                def exchange_batch(b):
                    # ship this batch's ctx^T; chunk j (128 q cols) -> core j
                    nc.sync.dma_start(
                        out=a2a_in[b][:].rearrange("(j p) t -> p j t", p=FPC),
                        in_=ctx_sb[:, b * TQ : (b + 1) * TQ].rearrange(
                            "p (j t) -> p j t", t=TQ // 8
                        ),
                    )
                    if n_cores > 1:
                        nc.gpsimd.collective_compute(
                            "AllToAll",
                            ALU.bypass,
                            replica_groups=[list(range(n_cores))],
                            ins=[a2a_in[b][:]],
                            outs=[a2a_out[b][:]],
                        )
                    else:
                        # single-core profiling variant: plain copy instead
                        nc.sync.dma_start(out=a2a_out[b][:], in_=a2a_in[b][:])

                pc = tc.alloc_tile_pool(name="pc", bufs=3)
                pc_ps = tc.alloc_tile_pool(name="pc_ps", bufs=2, space="PSUM")
                pc_small = tc.alloc_tile_pool(name="pc_small", bufs=4)

                def phase_c_mc(mc):
                    # output projection + residual + LayerNorm for this
                    # core's 128 q rows of batch mc
                    ps_o = [
                        pc_ps.tile([128, 512], f32, tag="ps_o",
                                   name=f"ps_o{mc}_{nn_}")
                        for nn_ in range(2)
                    ]
                    for kc in range(D // 128):
                        lhs = pc.tile([128, 128], pdt, tag="octx")
                        nc.sync.dma_start(
                            out=lhs,
                            in_=a2a_out[mc][kc * 128 : (kc + 1) * 128, :],
                        )
                        for nn in range(2):
                            nc.tensor.matmul(
                                ps_o[nn],
                                lhs,
                                wo_sb[:, kc, nn * 512 : (nn + 1) * 512],
                                start=(kc == 0),
                                stop=(kc == D // 128 - 1),
                            )
                    o_sb = pc.tile([128, D], f32, tag="o_sb")
                    for nn in range(2):
                        nc.vector.tensor_add(
                            out=o_sb[:, nn * 512 : (nn + 1) * 512],
                            in0=ps_o[nn],
                            in1=qres_sb[:, mc, nn * 512 : (nn + 1) * 512],
                        )
                    # LayerNorm over the free (feature) dim
                    stats = pc_small.tile([128, 2, 6], f32, tag="stats")
                    for sg in range(2):
                        nc.vector.bn_stats(
                            out=stats[:, sg, :], in_=o_sb[:, sg * 512 : (sg + 1) * 512]
                        )
                    mv = pc_small.tile([128, 2], f32, tag="mv")
                    nc.vector.bn_aggr(out=mv, in_=stats)
                    mean, var = mv[:, 0:1], mv[:, 1:2]
                    xve = pc_small.tile([128, 1], f32, tag="xve")
                    nc.vector.tensor_scalar_add(out=xve, in0=var, scalar1=eps_sb)
                    std = pc_small.tile([128, 1], f32, tag="std")
                    nc.scalar.activation(out=std, in_=var, func=Sqrt, bias=eps_sb)
                    rstd = pc_small.tile([128, 1], f32, tag="rstd")
                    nc.vector.reciprocal(rstd, std)
                    # one Newton step for rsqrt accuracy:
                    # r <- r * (1.5 - 0.5 * x * r^2)
                    tnw = pc_small.tile([128, 1], f32, tag="tnw")
                    nc.vector.tensor_mul(out=tnw, in0=rstd, in1=rstd)
                    nc.vector.tensor_mul(out=tnw, in0=tnw, in1=xve)
                    nc.vector.tensor_scalar(
                        out=tnw, in0=tnw, scalar1=-0.5, scalar2=1.5,
                        op0=ALU.mult, op1=ALU.add,
                    )
                    nc.vector.tensor_scalar_mul(out=rstd, in0=rstd, scalar1=tnw)
                    nc.vector.tensor_scalar(
                        out=o_sb, in0=o_sb, scalar1=mean, scalar2=rstd,
                        op0=ALU.subtract, op1=ALU.mult,
                    )
                    nc.vector.tensor_mul(out=o_sb, in0=o_sb, in1=gamma_sb)
                    nc.vector.tensor_add(out=o_sb, in0=o_sb, in1=beta_sb)
                    nc.sync.dma_start(
                        out=out[mc * 128 : (mc + 1) * 128, :], in_=o_sb
                    )

                pairs = [(0, 0, 0), (1, 0, 1), (2, 1, 0), (3, 1, 1)]
                pending = None
                for idx, (pi, b, hh) in enumerate(pairs):
                    if idx == 0:
                        for t in range(4):
                            for th in bd_raw_stages(pi, b, hh, t):
                                th()
                    if idx == 3:
                        # batch-0 output projection overlaps batch-1 attention
                        phase_c_mc(0)
                    fill0 = []
                    for t in range(4):
                        fill0 += bd_raw_stages(pi, b, hh, 4 + t)
                    pending = attn_half(pi, b, hh, 0, fill0, pending)
                    fill1 = []
                    if idx + 1 < 4:
                        pj, bj, hj = pairs[idx + 1]
                        for t in range(4):
                            fill1 += bd_raw_stages(pj, bj, hj, t)
                    pending = attn_half(pi, b, hh, 1, fill1, pending)
                    if idx == 1 or idx == 3:
                        pending()
                        pending = None
                        normalize_batch(b)
                        exchange_batch(b)

                phase_c_mc(1)

                pc_small.release()
                pc_ps.release()
                pc.release()
                pb_ctx.release()
                pb_ps2.release()
                pb_ps.release()
                pb_small.release()
                pb_bc.release()
                pb_prod.release()
                pb_eac.release()
                pb_bdt.release()
                pb_rows.release()
    return nc


def _make_in_maps(inputs, mm_dtype="bfloat16"):
    query = np.asarray(inputs["query"], np.float32)
    key_value = np.asarray(inputs["key_value"], np.float32)
    relative = np.asarray(inputs["relative"], np.float32)
    content_bias = np.asarray(inputs["content_bias"], np.float32)
    position_bias = np.asarray(inputs["position_bias"], np.float32)
    Wq, Wk = np.asarray(inputs["Wq"], np.float32), np.asarray(inputs["Wk"], np.float32)
    Wv, Wr = np.asarray(inputs["Wv"], np.float32), np.asarray(inputs["Wr"], np.float32)
    Wo = np.ascontiguousarray(np.asarray(inputs["Wo"], np.float32))
    ln_gamma = np.asarray(inputs["ln_gamma"], np.float32)
    ln_beta = np.asarray(inputs["ln_beta"], np.float32)

    qflat = query.reshape(B * TQ, D)
    if mm_dtype == "bfloat16":
        import ml_dtypes

        mdt = ml_dtypes.bfloat16
    else:
        mdt = np.float32
    xqT = np.ascontiguousarray(qflat.T).astype(mdt)
    xkvT = np.ascontiguousarray(key_value.reshape(B * TKV, D).T).astype(mdt)
    xrT = np.ascontiguousarray(relative.reshape(B * TKV, D).T).astype(mdt)
    Wq, Wk = Wq.astype(mdt), Wk.astype(mdt)
    Wv, Wr = Wv.astype(mdt), Wr.astype(mdt)
    Wo = Wo.astype(mdt)
    cb = content_bias.reshape(NH, DV)
    pb = position_bias.reshape(NH, DV)

    in_maps = []
    for c in range(N_CORES):
        fs = slice(FPC * c, FPC * (c + 1))
        in_maps.append(
            {
                "xqT": xqT,
                "xkvT": xkvT,
                "xrT": xrT,
                "wq": np.ascontiguousarray(Wq[:, fs]),
                "wk": np.ascontiguousarray(Wk[:, fs]),
                "wv": np.ascontiguousarray(Wv[:, fs]),
                "wr": np.ascontiguousarray(Wr[:, fs]),
                "wo": Wo,
                "cbv": np.ascontiguousarray(
                    cb[HPC * c : HPC * (c + 1)].reshape(FPC, 1)
                ),
                "pbv": np.ascontiguousarray(
                    pb[HPC * c : HPC * (c + 1)].reshape(FPC, 1)
                ),
                "qres": np.ascontiguousarray(qflat[RPC * c : RPC * (c + 1)]),
                "gamma": ln_gamma,
                "beta": ln_beta,
            }
        )
    return in_maps


def run_on_hw(inputs, trace=False, score_dtype="bfloat16", proj_dtype="bfloat16"):
    from concourse.bass_utils import run_bass_kernel_spmd

    key = (score_dtype, proj_dtype)
    nc = _CACHE.get(key)
    if nc is None:
        nc = build_program(score_dtype=score_dtype, proj_dtype=proj_dtype)
        _CACHE[key] = nc
    in_maps = _make_in_maps(inputs, mm_dtype=proj_dtype)
    res = run_bass_kernel_spmd(nc, in_maps, list(range(N_CORES)), trace=trace)
    outs = np.concatenate(
        [np.asarray(res.results[c]["out"]) for c in range(N_CORES)], axis=0
    )
    return outs.reshape(B, TQ, D), res


def kernel(**inputs) -> np.ndarray:
    out, _ = run_on_hw(inputs)
    return out
